# revision 12
# baseline (speedup 1.0000x reference)
"""CSplineBasic Trainium2 kernel: conditional rational-quadratic spline flow.

Strategy (pure data-parallel over batch, 8 cores):
  - Host precomputes weight-only transforms: the fa/fm conditioner MLPs are
    fused (fa_W2@fm_W0), the final fm_W2 projection and the f1/f2 first-layer
    contraction over the 129 conditioner outputs are factorized into
    per-column-slab matrices U_j[(g,f),:] = fm_W2[f]*V12[4j+g,:], with the
    null-token masking and the constant v-column folded into biases.
  - Device evaluates the scalar conditioner g(c) for all B*128 elements as
    fp32r matmuls (4 col-groups x 32 features on partitions, rows on free),
    accumulating the f1/f2 first-layer pre-activations directly in PSUM.
  - f1 -> spline1 (row-major searchsorted via compare+reduce, one-hot
    gathers, rational-quadratic formula) -> f2 -> spline2 -> outputs.
"""
import sys
for _p in ('/opt/trn_rl_repo', '/root/.axon_site/_ro/trn_rl_repo'):
    if _p not in sys.path:
        sys.path.append(_p)

from contextlib import ExitStack
import numpy as np

import concourse.bass as bass
import concourse.tile as tile
from concourse import bacc
import concourse.mybir as mybir

F32 = mybir.dt.float32
F32R = mybir.dt.float32r
AF = mybir.ActivationFunctionType
ALU = mybir.AluOpType
AX = mybir.AxisListType

B = 16384
NCORES = 8
RC = B // NCORES          # rows per core (2048)
NCH = RC // 128           # row chunks (16)
COND = 128
KNOT = 10
BINT = 5.0
HID = 64


# ---------------------------------------------------------------------------
# host-side constant building (weights only)
# ---------------------------------------------------------------------------
def build_consts(inp):
    f64 = np.float64
    faW = [np.asarray(inp[f'fa_W{i}'], f64) for i in range(3)]
    fab = [np.asarray(inp[f'fa_b{i}'], f64) for i in range(3)]
    fmW = [np.asarray(inp[f'fm_W{i}'], f64) for i in range(3)]
    fmb = [np.asarray(inp[f'fm_b{i}'], f64) for i in range(3)]
    f1W = [np.asarray(inp[f'f1_W{i}'], f64) for i in range(3)]
    f1b = [np.asarray(inp[f'f1_b{i}'], f64) for i in range(3)]
    f2W = [np.asarray(inp[f'f2_W{i}'], f64) for i in range(3)]
    f2b = [np.asarray(inp[f'f2_b{i}'], f64) for i in range(3)]
    ip = int(inp['index_p'])
    iv = int(inp['index_v'])

    def ffn(z, Ws, bs):
        for i in range(len(Ws)):
            z = z @ Ws[i] + bs[i]
            if i < len(Ws) - 1:
                z = np.maximum(z, 0)
        return z

    Wf = faW[2] @ fmW[0]
    bf = fab[2] @ fmW[0] + fmb[0]
    kappa = ffn(np.asarray(inp['null_token'], f64), fmW, fmb)[0, 0]
    nu = ffn(ffn(np.array([[np.sin(f64(iv))]]), faW, fab), fmW, fmb)[0, 0]

    V12 = np.concatenate([f1W[0][1:COND + 1], f2W[0][1:COND + 1]], axis=1)  # (128,128)
    mask = np.ones(COND, bool)
    mask[ip] = False
    mask[ip + COND // 2] = False

    c = {}
    # L1 selector weights: w0sel[j, p, 32g+f] = fa_W0[0,f] * [p == 4j+g]
    w0sel = np.zeros((32, 128, 128), np.float32)
    for j in range(32):
        for g in range(4):
            w0sel[j, 4 * j + g, 32 * g:32 * g + 32] = faW[0][0].astype(np.float32)
    c['w0sel'] = np.ascontiguousarray(w0sel.transpose(1, 0, 2))
    # U matrices: ublk[j, 32g+f, m] = fm_W2[f,0] * V12[4j+g, m]  (masked cols zero)
    ublk = np.zeros((32, 128, 128), np.float32)
    for j in range(32):
        for g in range(4):
            cc = 4 * j + g
            if mask[cc]:
                ublk[j, 32 * g:32 * g + 32, :] = np.outer(fmW[2][:, 0], V12[cc]).astype(np.float32)
    c['ublk'] = np.ascontiguousarray(ublk.transpose(1, 0, 2))

    def blockdiag4(W):
        out = np.zeros((128, 128), np.float32)
        for g in range(4):
            out[32 * g:32 * g + 32, 32 * g:32 * g + 32] = W.astype(np.float32)
        return out
    c['wl2'] = blockdiag4(faW[1])
    c['wl3'] = blockdiag4(Wf)
    c['wl4'] = blockdiag4(fmW[1])

    Vsum = V12[mask].sum(0)
    b0eff = np.concatenate([
        nu * f1W[0][COND + 1] + f1b[0] + kappa * (V12[ip, :64] + V12[ip + 64, :64]),
        nu * f2W[0][COND + 1] + f2b[0] + kappa * (V12[ip, 64:] + V12[ip + 64, 64:]),
    ]) + fmb[2][0] * Vsum

    bcol = np.zeros((128, 8), np.float32)
    bcol[:, 0] = np.tile(fab[0], 4)
    bcol[:, 1] = np.tile(fab[1], 4)
    bcol[:, 2] = np.tile(bf, 4)
    bcol[:, 3] = np.tile(fmb[1], 4)
    bcol[:, 4] = b0eff.astype(np.float32)
    bcol[0:64, 5] = f1b[1].astype(np.float32)
    bcol[0:64, 6] = f2b[1].astype(np.float32)
    c['bcol'] = bcol

    w1x = np.zeros((1, 128), np.float32)
    w1x[0, 0:64] = f1W[0][0].astype(np.float32)
    c['w1x'] = w1x
    w2x = np.zeros((1, 128), np.float32)
    w2x[0, 64:128] = f2W[0][0].astype(np.float32)
    c['w2x'] = w2x

    c['w1f1'] = f1W[1].astype(np.float32)                     # (64,64)
    w1f2 = np.zeros((128, 64), np.float32)
    w1f2[64:128] = f2W[1].astype(np.float32)
    c['w1f2'] = w1f2

    def wbmat(W2, b2):
        out = np.zeros((65, 32), np.float32)
        out[0:64, 0:29] = W2.astype(np.float32)
        out[64, 0:29] = b2.astype(np.float32)
        return out
    c['wb1'] = wbmat(f1W[2], f1b[2])
    c['wb2'] = wbmat(f2W[2], f2b[2])

    c['ident'] = np.eye(128, dtype=np.float32)
    iota = np.zeros((128, 24), np.float32)
    iota[:, 0:12] = np.arange(12)
    iota[:, 12:24] = np.arange(12) + 1
    c['iota'] = iota
    sm = np.ones(320, np.float32)
    sm[::10] = 0.0
    c['smask'] = np.tile(sm[None, :], (128, 1))
    gp = np.zeros(36, np.float32)
    gp[0] = -BINT          # w slot 0
    gp[12] = -BINT         # h slot 0
    gp[24] = 1.0           # d slot 0
    gp[34] = 1.0           # d slot 10
    c['gpat'] = np.tile(gp[None, :], (128, 1))
    return c


CONST_DTYPES = {
    'w0sel': F32R, 'ublk': F32R, 'wl2': F32R, 'wl3': F32R, 'wl4': F32R,
    'w1x': F32R, 'w2x': F32R, 'w1f1': F32R, 'w1f2': F32R,
    'wb1': F32R, 'wb2': F32R,
    'bcol': F32, 'iota': F32, 'smask': F32,
}




# blob packing: (name, partition_rows, free_elems) in order
BLOB_R_SPEC = [
    ('w0sel', 128, 32 * 128), ('ublk', 128, 32 * 128),
    ('wl2', 128, 128), ('wl3', 128, 128), ('wl4', 128, 128),
    ('w1x', 1, 128), ('w2x', 1, 128), ('w1f1', 64, 64), ('w1f2', 128, 64),
    ('wb1', 65, 32), ('wb2', 65, 32), ('ident', 128, 128),
    ('c_nat', 128, 16 * 128), ('x11t', 1, 2048),
]
BLOB_F_SPEC = [
    ('bcol', 128, 8), ('iota', 128, 24), ('smask', 128, 320),
    ('x_row', 128, 32), ('gpat', 128, 36),
]
BLOB_R_OFF = {}
_o = 0
for _n, _p, _f in BLOB_R_SPEC:
    BLOB_R_OFF[_n] = _o; _o += _f
BLOB_R_LEN = _o
BLOB_F_OFF = {}
_o = 0
for _n, _p, _f in BLOB_F_SPEC:
    BLOB_F_OFF[_n] = _o; _o += _f
BLOB_F_LEN = _o


def pack_blobs(consts, c_shard, x_shard):
    br = np.zeros((128, BLOB_R_LEN), np.float32)
    def put_r(name, arr):
        o = BLOB_R_OFF[name]
        arr = np.asarray(arr, np.float32)
        p = arr.shape[0]
        br[:p, o:o + int(np.prod(arr.shape[1:]))] = arr.reshape(p, -1)
    for n in ('w0sel', 'ublk', 'wl2', 'wl3', 'wl4', 'w1x', 'w2x', 'w1f1',
              'wb1', 'wb2', 'ident'):
        put_r(n, consts[n])
    br[64:128, BLOB_R_OFF['w1f2']:BLOB_R_OFF['w1f2'] + 64] = consts['w1f2'][64:128]
    cn = c_shard.reshape(16, 128, 128).transpose(1, 0, 2).reshape(128, 2048)
    br[:, BLOB_R_OFF['c_nat']:BLOB_R_OFF['c_nat'] + 2048] = cn
    br[0, BLOB_R_OFF['x11t']:BLOB_R_OFF['x11t'] + 2048] = x_shard[:, 0]
    bf = np.zeros((128, BLOB_F_LEN), np.float32)
    def put_f(name, arr):
        o = BLOB_F_OFF[name]
        arr = np.asarray(arr, np.float32)
        p = arr.shape[0]
        bf[:p, o:o + int(np.prod(arr.shape[1:]))] = arr.reshape(p, -1)
    put_f('bcol', consts['bcol'])
    put_f('iota', consts['iota'])
    put_f('smask', consts['smask'])
    put_f('x_row', x_shard.reshape(16, 128, 2).transpose(1, 0, 2).reshape(128, 32))
    put_f('gpat', consts['gpat'])
    return br, bf


def _bc(ap, pos, n):
    """insert a step-0 broadcast dim of size n at position pos of an AP"""
    lst = [list(d) for d in ap.ap]
    lst.insert(pos, [0, n])
    return bass.AP(tensor=ap.tensor, offset=ap.offset, ap=lst)


# ---------------------------------------------------------------------------
# the bass program
# ---------------------------------------------------------------------------
def build_program():
    nc = bacc.Bacc("TRN2", target_bir_lowering=False)

    d_br = nc.dram_tensor("blob_r", [128, BLOB_R_LEN], F32, kind="ExternalInput")
    d_bf = nc.dram_tensor("blob_f", [128, BLOB_F_LEN], F32, kind="ExternalInput")
    d_ox3 = nc.dram_tensor("o_x3", [128, NCH, 2], F32, kind="ExternalOutput")
    d_oja = nc.dram_tensor("o_ja", [128, NCH], F32, kind="ExternalOutput")

    with ExitStack() as ctx:
        tc = ctx.enter_context(tile.TileContext(nc))
        p_const = ctx.enter_context(tc.tile_pool(name="const", bufs=1))
        p_cdat = ctx.enter_context(tc.tile_pool(name="cdat", bufs=1))
        p_acts = ctx.enter_context(tc.tile_pool(name="acts", bufs=2))
        p_hs = ctx.enter_context(tc.tile_pool(name="hs", bufs=1))
        p_spl = ctx.enter_context(tc.tile_pool(name="spl", bufs=2))
        p_frm = ctx.enter_context(tc.tile_pool(name="frm", bufs=2))
        p_out = ctx.enter_context(tc.tile_pool(name="out", bufs=1))
        ps_w = ctx.enter_context(tc.tile_pool(name="psw", bufs=2, space="PSUM"))
        ps_p12 = ctx.enter_context(tc.tile_pool(name="psp12", bufs=1, space="PSUM"))
        ps_misc = ctx.enter_context(tc.tile_pool(name="psmisc", bufs=1, space="PSUM"))

        # ---- load blobs (2 DMAs; fp32r blob cast on the way in) ----
        blob_r = p_const.tile([128, BLOB_R_LEN], F32R, tag="blob_r")
        nc.gpsimd.dma_start(blob_r[:], d_br[:, :])
        blob_f = p_const.tile([128, BLOB_F_LEN], F32, tag="blob_f")
        nc.gpsimd.dma_start(blob_f[:], d_bf[:, :])

        # tiny observer ops: absorb the two DMA-lane sem ticks into each
        # engine's vector clock once, so no real instruction needs >1 wait
        dob = p_const.tile([1, 8], F32, tag="dob")
        dobr = p_const.tile([1, 8], F32R, tag="dobr")
        nc.scalar.copy(dob[0:1, 0:1], blob_f[0:1, 0:1])
        nc.vector.tensor_copy(dob[0:1, 1:2], blob_f[0:1, 0:1])
        nc.vector.tensor_copy(dobr[0:1, 1:2], blob_r[0:1, 0:1])
        nc.scalar.copy(dobr[0:1, 2:3], blob_r[0:1, 0:1])

        def vr(name, prows=128):
            o = BLOB_R_OFF[name]
            fl = next(s for s in BLOB_R_SPEC if s[0] == name)[2]
            return blob_r[0:prows, o:o + fl]

        def vf(name, prows=128):
            o = BLOB_F_OFF[name]
            fl = next(s for s in BLOB_F_SPEC if s[0] == name)[2]
            return blob_f[0:prows, o:o + fl]

        ct = {}
        ct['w0sel'] = vr('w0sel').rearrange("p (j m) -> p j m", j=32)
        ct['ublk'] = vr('ublk').rearrange("p (j m) -> p j m", j=32)
        for k in ('wl2', 'wl3', 'wl4'):
            ct[k] = vr(k)
        ct['w1x'] = vr('w1x', 1)
        ct['w2x'] = vr('w2x', 1)
        ct['w1f1'] = vr('w1f1', 64)
        ct['w1f2'] = blob_r[:, BLOB_R_OFF['w1f2']:BLOB_R_OFF['w1f2'] + 64]
        ct['wb1'] = vr('wb1', 65)
        ct['wb2'] = vr('wb2', 65)
        ident_r = vr('ident')
        ct['bcol'] = vf('bcol')
        ct['iota'] = vf('iota')
        ct['smask'] = vf('smask')
        c_nat = vr('c_nat').rearrange("p (a k) -> p a k", a=NCH)
        x11t = blob_r[0:1, BLOB_R_OFF['x11t']:BLOB_R_OFF['x11t'] + RC]
        x_row = vf('x_row').rearrange("p (a t) -> p a t", t=2)
        gpat = vf('gpat')

        # ---- stage 1: transpose c -> cT (cols on partitions, rows on free) ----
        cT = p_cdat.tile([128, RC], F32R, tag="cT")
        for i in range(NCH):
            pt = ps_misc.tile([128, 128], F32R, tag="misc")
            nc.tensor.transpose(pt[:], c_nat[:, i, :], ident_r[:])
            nc.vector.tensor_copy(cT[:, 128 * i:128 * (i + 1)], pt[:])

        # ---- stage 2: conditioner MLP, fused first-layer contraction ----
        NB = 2                   # row blocks
        RB = RC // NB            # rows per block (1024)
        NH = RB // 512           # 512-halves per block
        p12s = []
        for b in range(NB):
            p12 = ps_p12.tile([128, RB], F32, tag="p12")
            p12s.append(p12)
            r0 = b * RB
            for j in range(32):
                psa = ps_w.tile([128, RB], F32, tag="w")
                for h in range(NH):
                    nc.tensor.matmul(psa[:, 512 * h:512 * (h + 1)],
                                     ct['w0sel'][:, j, :],
                                     cT[:, r0 + 512 * h:r0 + 512 * (h + 1)],
                                     start=True, stop=True)
                a1 = p_acts.tile([128, RB], F32R, tag="a1")
                nc.scalar.activation(a1[:], psa[:], AF.Relu, bias=ct['bcol'][:, 0:1])

                psb = ps_w.tile([128, RB], F32, tag="w")
                for h in range(NH):
                    nc.tensor.matmul(psb[:, 512 * h:512 * (h + 1)], ct['wl2'][:],
                                     a1[:, 512 * h:512 * (h + 1)], start=True, stop=True)
                a2 = p_acts.tile([128, RB], F32R, tag="a2")
                nc.vector.tensor_scalar(a2[:], psb[:], ct['bcol'][:, 1:2], 0.0, ALU.add, ALU.max)

                psc = ps_w.tile([128, RB], F32, tag="w")
                for h in range(NH):
                    nc.tensor.matmul(psc[:, 512 * h:512 * (h + 1)], ct['wl3'][:],
                                     a2[:, 512 * h:512 * (h + 1)], start=True, stop=True)
                m1 = p_acts.tile([128, RB], F32R, tag="m1")
                nc.scalar.activation(m1[:], psc[:], AF.Relu, bias=ct['bcol'][:, 2:3])

                psd = ps_w.tile([128, RB], F32, tag="w")
                for h in range(NH):
                    nc.tensor.matmul(psd[:, 512 * h:512 * (h + 1)], ct['wl4'][:],
                                     m1[:, 512 * h:512 * (h + 1)], start=True, stop=True)
                m2 = p_acts.tile([128, RB], F32R, tag="m2")
                nc.vector.tensor_scalar(m2[:], psd[:], ct['bcol'][:, 3:4], 0.0, ALU.add, ALU.max)

                for h in range(NH):
                    nc.tensor.matmul(p12[:, 512 * h:512 * (h + 1)], ct['ublk'][:, j, :],
                                     m2[:, 512 * h:512 * (h + 1)],
                                     start=(j == 0), stop=False)
            # x11 contribution (K=1, full 128-partition lhsT, closes the group)
            for h in range(NH):
                nc.tensor.matmul(p12[:, 512 * h:512 * (h + 1)], ct['w1x'][:],
                                 x11t[:, r0 + 512 * h:r0 + 512 * (h + 1)],
                                 start=False, stop=True)

        # ---- stage 3: f1 chain + p2c stash ----
        h1_1 = p_hs.tile([64, RC], F32R, tag="h1_1")
        p2c = p_hs.tile([128, RC], F32, tag="p2c")
        h2_1 = p_hs.tile([65, RC], F32R, tag="h2_1")
        h2_2 = p_hs.tile([65, RC], F32R, tag="h2_2")
        ones_b = _bc(ct['iota'][64:65, 12:13], 2, RC)[:, 0, :]
        nc.vector.tensor_copy(h2_1[64:65, :], ones_b)
        nc.vector.tensor_copy(h2_2[64:65, :], ones_b)
        for b in range(NB):
            sl = slice(b * RB, (b + 1) * RB)
            nc.scalar.activation(h1_1[:, sl], p12s[b][0:64, :], AF.Relu, bias=ct['bcol'][0:64, 4:5])
            nc.scalar.activation(p2c[64:128, sl], p12s[b][64:128, :], AF.Identity, bias=ct['bcol'][64:128, 4:5])
        for b in range(NB):
            psh = ps_w.tile([64, RB], F32, tag="w")
            for h in range(NH):
                nc.tensor.matmul(psh[:, 512 * h:512 * (h + 1)], ct['w1f1'][:],
                                 h1_1[:, b * RB + 512 * h:b * RB + 512 * (h + 1)],
                                 start=True, stop=True)
            nc.scalar.activation(h2_1[0:64, b * RB:(b + 1) * RB], psh[:], AF.Relu,
                                 bias=ct['bcol'][0:64, 5:6])

        p1ps = ps_misc.tile([128, NCH, 32], F32, tag="spl")
        for i in range(NCH):
            nc.tensor.matmul(p1ps[:, i, :], h2_1[:, 128 * i:128 * (i + 1)], ct['wb1'][:],
                             start=True, stop=True)

        # ---- spline evaluation (row-major) ----
        def spline(pps, x_in, ytile, pdtile, sidx):
            # pps: psum (128, NCH, 32); x_in: (128, NCH, 1) view
            E = p_spl.tile([128, NCH * 20], F32, tag="E")
            E3 = E[:].rearrange("p (a k) -> p a k", k=20)
            nc.scalar.activation(E3, pps[:, :, 0:20], AF.Exp)
            SpE = p_spl.tile([128, NCH, 9], F32, tag="SpE")
            nc.scalar.activation(SpE[:], pps[:, :, 20:29], AF.Exp)
            SpP = p_spl.tile([128, NCH, 9], F32, tag="SpP")
            nc.vector.tensor_scalar(SpP[:], SpE[:], 1.0, None, ALU.add)
            SpD = p_spl.tile([128, NCH, 9], F32, tag="SpD")
            nc.scalar.activation(SpD[:], SpP[:], AF.Ln)
            CS = p_spl.tile([128, NCH * 20], F32, tag="CS")
            nc.vector.tensor_tensor_scan(CS[:], ct['smask'][:], E[:], 0.0, ALU.mult, ALU.add)
            CS3 = CS[:].rearrange("p (a k) -> p a k", k=20)
            R2 = p_spl.tile([128, NCH, 2], F32, tag="R2")
            nc.vector.reciprocal(R2[:, :, 0:1], CS3[:, :, 9:10])
            nc.vector.reciprocal(R2[:, :, 1:2], CS3[:, :, 19:20])

            GAT = p_spl.tile([128, NCH, 3, 12], F32, tag="GAT")
            nc.vector.tensor_copy(GAT[:].rearrange("p a b c -> p a (b c)"), _bc(gpat, 1, NCH))
            TMPW = p_spl.tile([128, NCH, 10], F32, tag="TMPW")
            nc.vector.tensor_tensor(TMPW[:], CS3[:, :, 0:10], _bc(R2[:, :, 0:1], 2, 10), ALU.mult)
            nc.vector.tensor_scalar(GAT[:, :, 0, 1:11], TMPW[:], 2 * BINT, -BINT, ALU.mult, ALU.add)
            TMPH = p_spl.tile([128, NCH, 10], F32, tag="TMPH")
            nc.vector.tensor_tensor(TMPH[:], CS3[:, :, 10:20], _bc(R2[:, :, 1:2], 2, 10), ALU.mult)
            nc.vector.tensor_scalar(GAT[:, :, 1, 1:11], TMPH[:], 2 * BINT, -BINT, ALU.mult, ALU.add)
            nc.vector.tensor_scalar(GAT[:, :, 2, 1:10], SpD[:], 0.001, None, ALU.add)

            CMP = p_spl.tile([128, NCH, 11], F32, tag="CMP")
            nc.vector.tensor_tensor(CMP[:], GAT[:, :, 0, 0:11], _bc(x_in, 2, 11), ALU.is_le)
            IDX = p_frm.tile([128, NCH], F32, tag=f"IDX{sidx}")
            nc.vector.tensor_reduce(IDX[:], CMP[:], AX.X, ALU.add)
            nc.vector.tensor_scalar(IDX[:], IDX[:], 1.0, 10.0, ALU.max, ALU.min)
            IDXb = _bc(IDX[:], 2, 12)
            OHR = p_spl.tile([128, NCH, 12], F32, tag="OHR")
            iot = ct['iota'][:, 0:12]
            nc.vector.tensor_tensor(OHR[:], _bc(iot, 1, NCH), IDXb, ALU.is_equal)
            OHL = p_spl.tile([128, NCH, 12], F32, tag="OHL")
            iop = ct['iota'][:, 12:24]
            nc.vector.tensor_tensor(OHL[:], _bc(iop, 1, NCH), IDXb, ALU.is_equal)

            TMP = p_spl.tile([128, NCH, 3, 12], F32, tag="TMP")
            PL = p_spl.tile([128, NCH, 3], F32, tag="PL")
            nc.vector.tensor_tensor(TMP[:], GAT[:], _bc(OHL[:], 2, 3), ALU.mult)
            nc.vector.tensor_reduce(PL[:], TMP[:], AX.X, ALU.add)
            PR = p_spl.tile([128, NCH, 3], F32, tag="PR")
            nc.vector.tensor_tensor(TMP[:], GAT[:], _bc(OHR[:], 2, 3), ALU.mult)
            nc.vector.tensor_reduce(PR[:], TMP[:], AX.X, ALU.add)

            def frm(tag):
                t = p_frm.tile([128, NCH], F32, tag=f"{tag}{sidx}")
                return t, t[:].rearrange("p (a k) -> p a k", k=1)
            wl = PL[:, :, 0:1]; hl = PL[:, :, 1:2]; dl = PL[:, :, 2:3]
            wr = PR[:, :, 0:1]; hr = PR[:, :, 1:2]; dr = PR[:, :, 2:3]
            DWH = p_spl.tile([128, NCH, 2], F32, tag="DWH")
            nc.vector.tensor_tensor(DWH[:], PR[:, :, 0:2], PL[:, :, 0:2], ALU.subtract)
            dh = DWH[:, :, 1:2]
            RDW, RDWv = frm("RDW")
            nc.vector.reciprocal(RDWv, DWH[:, :, 0:1])
            T0, T0v = frm("T0")
            nc.vector.tensor_tensor(T0v, x_in, wl, ALU.subtract)
            T, Tv = frm("T")
            nc.vector.tensor_tensor(Tv, T0v, RDWv, ALU.mult)
            S, Sv = frm("S")
            nc.vector.tensor_tensor(Sv, dh, RDWv, ALU.mult)
            T1, T1v = frm("T1")
            nc.vector.tensor_scalar(T1v, Tv, -1.0, 1.0, ALU.mult, ALU.add)
            U_, Uv = frm("U")
            nc.vector.tensor_tensor(Uv, Tv, T1v, ALU.mult)
            T2, T2v = frm("T2")
            nc.vector.tensor_tensor(T2v, Tv, Tv, ALU.mult)
            V1a, V1av = frm("V1a")
            nc.vector.tensor_tensor(V1av, Sv, T2v, ALU.mult)
            V1b, V1bv = frm("V1b")
            nc.vector.tensor_tensor(V1bv, dl, Uv, ALU.mult)
            V1, V1v = frm("V1")
            nc.vector.tensor_tensor(V1v, V1av, V1bv, ALU.add)
            A_, A_v = frm("A_")
            nc.vector.tensor_tensor(A_v, dr, dl, ALU.add)
            B_, B_v = frm("B_")
            nc.vector.scalar_tensor_tensor(B_v, Sv, -2.0, A_v, ALU.mult, ALU.add)
            C_, C_v = frm("C_")
            nc.vector.tensor_tensor(C_v, B_v, Uv, ALU.mult)
            V2, V2v = frm("V2")
            nc.vector.tensor_tensor(V2v, C_v, Sv, ALU.add)
            RV2, RV2v = frm("RV2")
            nc.vector.reciprocal(RV2v, V2v)
            Y1, Y1v = frm("Y1")
            nc.vector.tensor_tensor(Y1v, V1v, RV2v, ALU.mult)
            Y2, Y2v = frm("Y2")
            nc.vector.tensor_tensor(Y2v, Y1v, dh, ALU.mult)
            yv = ytile[:].rearrange("p (a k) -> p a k", k=1)
            nc.vector.tensor_tensor(yv, Y2v, hl, ALU.add)
            N1, N1v = frm("N1")
            nc.vector.tensor_tensor(N1v, dr, T2v, ALU.mult)
            N2, N2v = frm("N2")
            nc.vector.scalar_tensor_tensor(N2v, Sv, 2.0, Uv, ALU.mult, ALU.mult)
            T1S, T1Sv = frm("T1S")
            nc.vector.tensor_tensor(T1Sv, T1v, T1v, ALU.mult)
            N3, N3v = frm("N3")
            nc.vector.tensor_tensor(N3v, dl, T1Sv, ALU.mult)
            N12, N12v = frm("N12")
            nc.vector.tensor_tensor(N12v, N1v, N2v, ALU.add)
            NUM, NUMv = frm("NUM")
            nc.vector.tensor_tensor(NUMv, N12v, N3v, ALU.add)
            S2, S2v = frm("S2")
            nc.vector.tensor_tensor(S2v, Sv, Sv, ALU.mult)
            RVQ, RVQv = frm("RVQ")
            nc.vector.tensor_tensor(RVQv, RV2v, RV2v, ALU.mult)
            PDa, PDav = frm("PDa")
            nc.vector.tensor_tensor(PDav, S2v, NUMv, ALU.mult)
            pdv = pdtile[:].rearrange("p (a k) -> p a k", k=1)
            nc.vector.tensor_tensor(pdv, PDav, RVQv, ALU.mult)

        x22row = p_out.tile([128, NCH], F32, tag="x22row")
        pd1 = p_out.tile([128, NCH], F32, tag="pd1")
        spline(p1ps, x_row[:, :, 1:2], x22row, pd1, 1)

        # ---- x22 transpose -> x22t (1, RC) fp32r ----
        x22r = p_out.tile([128, NCH], F32R, tag="x22r")
        nc.vector.tensor_copy(x22r[:], x22row[:])
        ptx = ps_misc.tile([16, 128], F32R, tag="spl")
        nc.tensor.transpose(ptx[0:NCH, :], x22r[:, :], ident_r[:])
        xt16 = p_cdat.tile([16, 128], F32R, tag="xt16")
        nc.vector.tensor_copy(xt16[:], ptx[0:NCH, :])
        x22t = p_cdat.tile([1, RC], F32R, tag="x22t")
        nc.gpsimd.dma_start(x22t[:], xt16[:])

        # ---- f2 chain ----
        h1_2 = p_hs.tile([128, RC], F32R, tag="h1_2")
        for b in range(NB):
            psx = ps_w.tile([128, RB], F32, tag="w")
            for h in range(NH):
                nc.tensor.matmul(psx[:, 512 * h:512 * (h + 1)], ct['w2x'][:],
                                 x22t[:, b * RB + 512 * h:b * RB + 512 * (h + 1)],
                                 start=True, stop=True)
            sl = slice(b * RB, (b + 1) * RB)
            TS = p_hs.tile([128, RB], F32, tag="TS")
            nc.vector.tensor_tensor(TS[64:128, :], psx[64:128, :], p2c[64:128, sl], ALU.add)
            nc.vector.tensor_scalar(h1_2[64:128, sl], TS[64:128, :], 0.0, None, ALU.max)
        for b in range(NB):
            psh2 = ps_w.tile([64, RB], F32, tag="w")
            for h in range(NH):
                nc.tensor.matmul(psh2[:, 512 * h:512 * (h + 1)], ct['w1f2'][64:128, :],
                                 h1_2[64:128, b * RB + 512 * h:b * RB + 512 * (h + 1)],
                                 start=True, stop=True)
            nc.vector.tensor_scalar(h2_2[0:64, b * RB:(b + 1) * RB], psh2[:],
                                    ct['bcol'][0:64, 6:7], 0.0, ALU.add, ALU.max)

        p2ps = ps_misc.tile([128, NCH, 32], F32, tag="spl")
        for i in range(NCH):
            nc.tensor.matmul(p2ps[:, i, :], h2_2[:, 128 * i:128 * (i + 1)], ct['wb2'][:],
                             start=True, stop=True)

        x31row = p_out.tile([128, NCH], F32, tag="x31row")
        pd2 = p_out.tile([128, NCH], F32, tag="pd2")
        spline(p2ps, x_row[:, :, 0:1], x31row, pd2, 2)

        # ---- outputs ----
        x3row = p_out.tile([128, NCH, 2], F32, tag="x3row")
        nc.vector.tensor_copy(x3row[:, :, 0], x31row[:])
        nc.vector.tensor_copy(x3row[:, :, 1], x22row[:])
        jam = p_out.tile([128, NCH], F32, tag="jam")
        nc.vector.tensor_tensor(jam[:], pd1[:], pd2[:], ALU.mult)
        jarow = p_out.tile([128, NCH], F32, tag="jarow")
        nc.scalar.activation(jarow[:], jam[:], AF.Abs)
        nc.gpsimd.dma_start(d_ox3[:, :, :], x3row[:])
        nc.gpsimd.dma_start(d_oja[:, :], jarow[:])

    nc.compile()
    return nc


# ---------------------------------------------------------------------------
# entry point
# ---------------------------------------------------------------------------
_CACHE = {}


def kernel(**inputs):
    from concourse.bass_utils import run_bass_kernel_spmd

    consts = build_consts(inputs)
    x = np.ascontiguousarray(np.asarray(inputs['x'], np.float32))
    c = np.ascontiguousarray(np.asarray(inputs['c'], np.float32))

    if 'nc' not in _CACHE:
        _CACHE['nc'] = build_program()
    nc = _CACHE['nc']

    in_maps = []
    for m in range(NCORES):
        xs = x[m * RC:(m + 1) * RC]
        cs = c[m * RC:(m + 1) * RC]
        br, bf = pack_blobs(consts, cs, xs)
        in_maps.append({'blob_r': br, 'blob_f': bf})

    res = run_bass_kernel_spmd(nc, in_maps, core_ids=list(range(NCORES)))
    x3 = np.empty((B, 2), np.float32)
    ja = np.empty((B, 1), np.float32)
    for m in range(NCORES):
        r = res.results[m]
        x3[m * RC:(m + 1) * RC] = r['o_x3'].transpose(1, 0, 2).reshape(RC, 2)
        ja[m * RC:(m + 1) * RC] = r['o_ja'].transpose(1, 0).reshape(RC, 1)
    return x3, ja


if __name__ == "__main__":
    nc = build_program()
    print("program built ok")


# revision 20
# speedup vs baseline: 1.6111x; 1.6111x over previous
"""CSplineBasic Trainium2 kernel: conditional rational-quadratic spline flow.

Strategy (pure data-parallel over batch, 8 cores):
  - Host precomputes weight-only transforms: the fa/fm conditioner MLPs are
    fused (fa_W2@fm_W0), the final fm_W2 projection and the f1/f2 first-layer
    contraction over the 129 conditioner outputs are factorized into
    per-column-slab matrices U_j[(g,f),:] = fm_W2[f]*V12[4j+g,:], with the
    null-token masking and the constant v-column folded into biases.
  - Device evaluates the scalar conditioner g(c) for all B*128 elements as
    fp32r matmuls (4 col-groups x 32 features on partitions, rows on free),
    accumulating the f1/f2 first-layer pre-activations directly in PSUM.
  - f1 -> spline1 (row-major searchsorted via compare+reduce, one-hot
    gathers, rational-quadratic formula) -> f2 -> spline2 -> outputs.
"""
import sys
for _p in ('/opt/trn_rl_repo', '/root/.axon_site/_ro/trn_rl_repo'):
    if _p not in sys.path:
        sys.path.append(_p)

from contextlib import ExitStack
import numpy as np

import concourse.bass as bass
import concourse.tile as tile
from concourse import bacc
import concourse.mybir as mybir

F32 = mybir.dt.float32
F32R = mybir.dt.float32r
AF = mybir.ActivationFunctionType
ALU = mybir.AluOpType
AX = mybir.AxisListType

B = 16384
NCORES = 8
RC = B // NCORES          # rows per core (2048)
NCH = RC // 128           # row chunks (16)
COND = 128
KNOT = 10
BINT = 5.0
HID = 64


# ---------------------------------------------------------------------------
# host-side constant building (weights only)
# ---------------------------------------------------------------------------
def build_consts(inp):
    f64 = np.float64
    faW = [np.asarray(inp[f'fa_W{i}'], f64) for i in range(3)]
    fab = [np.asarray(inp[f'fa_b{i}'], f64) for i in range(3)]
    fmW = [np.asarray(inp[f'fm_W{i}'], f64) for i in range(3)]
    fmb = [np.asarray(inp[f'fm_b{i}'], f64) for i in range(3)]
    f1W = [np.asarray(inp[f'f1_W{i}'], f64) for i in range(3)]
    f1b = [np.asarray(inp[f'f1_b{i}'], f64) for i in range(3)]
    f2W = [np.asarray(inp[f'f2_W{i}'], f64) for i in range(3)]
    f2b = [np.asarray(inp[f'f2_b{i}'], f64) for i in range(3)]
    ip = int(inp['index_p'])
    iv = int(inp['index_v'])

    def ffn(z, Ws, bs):
        for i in range(len(Ws)):
            z = z @ Ws[i] + bs[i]
            if i < len(Ws) - 1:
                z = np.maximum(z, 0)
        return z

    Wf = faW[2] @ fmW[0]
    bf = fab[2] @ fmW[0] + fmb[0]
    kappa = ffn(np.asarray(inp['null_token'], f64), fmW, fmb)[0, 0]
    nu = ffn(ffn(np.array([[np.sin(f64(iv))]]), faW, fab), fmW, fmb)[0, 0]

    V12 = np.concatenate([f1W[0][1:COND + 1], f2W[0][1:COND + 1]], axis=1)  # (128,128)
    mask = np.ones(COND, bool)
    mask[ip] = False
    mask[ip + COND // 2] = False

    c = {}
    # L1 selector weights: w0sel[j, p, 32g+f] = fa_W0[0,f] * [p == 4j+g]
    w0sel = np.zeros((32, 128, 128), np.float32)
    for j in range(32):
        for g in range(4):
            w0sel[j, 4 * j + g, 32 * g:32 * g + 32] = faW[0][0].astype(np.float32)
    c['w0sel'] = np.ascontiguousarray(w0sel.transpose(1, 0, 2))
    # U matrices: ublk[j, 32g+f, m] = fm_W2[f,0] * V12[4j+g, m]  (masked cols zero)
    ublk = np.zeros((32, 128, 128), np.float32)
    for j in range(32):
        for g in range(4):
            cc = 4 * j + g
            if mask[cc]:
                ublk[j, 32 * g:32 * g + 32, :] = np.outer(fmW[2][:, 0], V12[cc]).astype(np.float32)
    c['ublk'] = np.ascontiguousarray(ublk.transpose(1, 0, 2))

    def blockdiag4(W):
        out = np.zeros((128, 128), np.float32)
        for g in range(4):
            out[32 * g:32 * g + 32, 32 * g:32 * g + 32] = W.astype(np.float32)
        return out
    c['wl2'] = blockdiag4(faW[1])
    c['wl3'] = blockdiag4(Wf)
    c['wl4'] = blockdiag4(fmW[1])

    Vsum = V12[mask].sum(0)
    b0eff = np.concatenate([
        nu * f1W[0][COND + 1] + f1b[0] + kappa * (V12[ip, :64] + V12[ip + 64, :64]),
        nu * f2W[0][COND + 1] + f2b[0] + kappa * (V12[ip, 64:] + V12[ip + 64, 64:]),
    ]) + fmb[2][0] * Vsum

    bcol = np.zeros((128, 8), np.float32)
    bcol[:, 0] = np.tile(fab[0], 4)
    bcol[:, 1] = np.tile(fab[1], 4)
    bcol[:, 2] = np.tile(bf, 4)
    bcol[:, 3] = np.tile(fmb[1], 4)
    bcol[:, 4] = b0eff.astype(np.float32)
    bcol[0:64, 5] = f1b[1].astype(np.float32)
    bcol[0:64, 6] = f2b[1].astype(np.float32)
    c['bcol'] = bcol

    w1x = np.zeros((1, 128), np.float32)
    w1x[0, 0:64] = f1W[0][0].astype(np.float32)
    c['w1x'] = w1x
    w2x = np.zeros((1, 128), np.float32)
    w2x[0, 64:128] = f2W[0][0].astype(np.float32)
    c['w2x'] = w2x

    c['w1f1'] = f1W[1].astype(np.float32)                     # (64,64)
    w1f2 = np.zeros((128, 64), np.float32)
    w1f2[64:128] = f2W[1].astype(np.float32)
    c['w1f2'] = w1f2

    def wbmat(W2, b2):
        out = np.zeros((65, 32), np.float32)
        out[0:64, 0:29] = W2.astype(np.float32)
        out[64, 0:29] = b2.astype(np.float32)
        return out
    c['wb1'] = wbmat(f1W[2], f1b[2])
    c['wb2'] = wbmat(f2W[2], f2b[2])

    c['ident'] = np.eye(128, dtype=np.float32)
    iota = np.zeros((128, 24), np.float32)
    iota[:, 0:12] = np.arange(12)
    iota[:, 12:24] = np.arange(12) + 1
    c['iota'] = iota
    sm = np.ones(320, np.float32)
    sm[::10] = 0.0
    c['smask'] = np.tile(sm[None, :], (128, 1))
    gp = np.zeros(36, np.float32)
    gp[0] = -BINT          # w slot 0
    gp[12] = -BINT         # h slot 0
    gp[24] = 1.0           # d slot 0
    gp[34] = 1.0           # d slot 10
    c['gpat'] = np.tile(gp[None, :], (128, 1))
    return c


CONST_DTYPES = {
    'w0sel': F32R, 'ublk': F32R, 'wl2': F32R, 'wl3': F32R, 'wl4': F32R,
    'w1x': F32R, 'w2x': F32R, 'w1f1': F32R, 'w1f2': F32R,
    'wb1': F32R, 'wb2': F32R,
    'bcol': F32, 'iota': F32, 'smask': F32,
}




# blob packing: (name, partition_rows, free_elems) in order
BLOB_R_SPEC = [
    ('c_nat', 128, 16 * 128), ('x11t', 1, 2048), ('ident', 128, 128),
    ('w0sel', 128, 32 * 128),
    ('wl2', 128, 128), ('wl3', 128, 128), ('wl4', 128, 128),
    ('w1x', 1, 128), ('w2x', 1, 128), ('w1f1', 64, 64), ('w1f2', 128, 64),
    ('wb1', 65, 32), ('wb2', 65, 32),
    ('ublk', 128, 32 * 128),
]
BLOB_F_SPEC = [
    ('bcol', 128, 8), ('iota', 128, 24), ('smask', 128, 320),
    ('x_row', 128, 32), ('gpat', 128, 36),
]
BLOB_R_OFF = {}
_o = 0
for _n, _p, _f in BLOB_R_SPEC:
    BLOB_R_OFF[_n] = _o; _o += _f
BLOB_R_LEN = _o
BLOB_F_OFF = {}
_o = 0
for _n, _p, _f in BLOB_F_SPEC:
    BLOB_F_OFF[_n] = _o; _o += _f
BLOB_F_LEN = _o


def pack_blobs(consts, c_shard, x_shard):
    br = np.zeros((128, BLOB_R_LEN), np.float32)
    def put_r(name, arr):
        o = BLOB_R_OFF[name]
        arr = np.asarray(arr, np.float32)
        p = arr.shape[0]
        br[:p, o:o + int(np.prod(arr.shape[1:]))] = arr.reshape(p, -1)
    for n in ('w0sel', 'ublk', 'wl2', 'wl3', 'wl4', 'w1x', 'w2x', 'w1f1',
              'wb1', 'wb2', 'ident'):
        put_r(n, consts[n])
    br[64:128, BLOB_R_OFF['w1f2']:BLOB_R_OFF['w1f2'] + 64] = consts['w1f2'][64:128]
    cn = c_shard.reshape(16, 128, 128).transpose(1, 0, 2).reshape(128, 2048)
    br[:, BLOB_R_OFF['c_nat']:BLOB_R_OFF['c_nat'] + 2048] = cn
    br[0, BLOB_R_OFF['x11t']:BLOB_R_OFF['x11t'] + 2048] = x_shard[:, 0]
    bf = np.zeros((128, BLOB_F_LEN), np.float32)
    def put_f(name, arr):
        o = BLOB_F_OFF[name]
        arr = np.asarray(arr, np.float32)
        p = arr.shape[0]
        bf[:p, o:o + int(np.prod(arr.shape[1:]))] = arr.reshape(p, -1)
    put_f('bcol', consts['bcol'])
    put_f('iota', consts['iota'])
    put_f('smask', consts['smask'])
    put_f('x_row', x_shard.reshape(16, 128, 2).transpose(1, 0, 2).reshape(128, 32))
    put_f('gpat', consts['gpat'])
    return br, bf


def _bc(ap, pos, n):
    """insert a step-0 broadcast dim of size n at position pos of an AP"""
    lst = [list(d) for d in ap.ap]
    lst.insert(pos, [0, n])
    return bass.AP(tensor=ap.tensor, offset=ap.offset, ap=lst)


# ---------------------------------------------------------------------------
# the bass program
# ---------------------------------------------------------------------------
def build_program():
    nc = bacc.Bacc("TRN2", target_bir_lowering=False)

    d_br = nc.dram_tensor("blob_r", [128, BLOB_R_LEN], F32, kind="ExternalInput")
    d_bf = nc.dram_tensor("blob_f", [128, BLOB_F_LEN], F32, kind="ExternalInput")
    d_ox3 = nc.dram_tensor("o_x3", [128, NCH, 2], F32, kind="ExternalOutput")
    d_oja = nc.dram_tensor("o_ja", [128, NCH], F32, kind="ExternalOutput")

    with ExitStack() as ctx:
        tc = ctx.enter_context(tile.TileContext(nc))
        p_const = ctx.enter_context(tc.tile_pool(name="const", bufs=1))
        p_cdat = ctx.enter_context(tc.tile_pool(name="cdat", bufs=1))
        p_acts = ctx.enter_context(tc.tile_pool(name="acts", bufs=2))
        p_hs = ctx.enter_context(tc.tile_pool(name="hs", bufs=1))
        p_spl = ctx.enter_context(tc.tile_pool(name="spl", bufs=2))
        p_frm = ctx.enter_context(tc.tile_pool(name="frm", bufs=2))
        p_out = ctx.enter_context(tc.tile_pool(name="out", bufs=1))

        # ---- load blobs (2 DMAs; fp32r blob cast on the way in) ----
        blob_r = p_const.tile([128, BLOB_R_LEN], F32R, tag="blob_r")
        CUT1 = BLOB_R_OFF['w0sel']          # end of data+ident section
        CUT2 = BLOB_R_OFF['ublk']           # start of ublk section
        nc.gpsimd.dma_start(blob_r[:, 0:CUT1], d_br[:, 0:CUT1])
        nc.gpsimd.dma_start(blob_r[:, CUT1:CUT2], d_br[:, CUT1:CUT2])
        nc.gpsimd.dma_start(blob_r[:, CUT2:], d_br[:, CUT2:])
        blob_f = p_const.tile([128, BLOB_F_LEN], F32, tag="blob_f")
        nc.gpsimd.dma_start(blob_f[:], d_bf[:, :])

        # tiny observer ops: absorb the two DMA-lane sem ticks into each
        # engine's vector clock once, so no real instruction needs >1 wait
        dob = p_const.tile([1, 8], F32, tag="dob")
        dobr = p_const.tile([1, 8], F32R, tag="dobr")
        nc.scalar.copy(dob[0:1, 0:1], blob_f[0:1, 0:1])
        nc.vector.tensor_copy(dob[0:1, 1:2], blob_f[0:1, 0:1])
        nc.vector.tensor_copy(dobr[0:1, 1:2], blob_r[0:1, 0:1])
        nc.scalar.copy(dobr[0:1, 2:3], blob_r[0:1, 0:1])
        nc.vector.tensor_copy(dobr[0:1, 3:4], blob_r[0:1, CUT1:CUT1 + 1])
        nc.scalar.copy(dobr[0:1, 4:5], blob_r[0:1, CUT1:CUT1 + 1])
        nc.vector.tensor_copy(dobr[0:1, 5:6], blob_r[0:1, CUT2:CUT2 + 1])
        nc.scalar.copy(dobr[0:1, 6:7], blob_r[0:1, CUT2:CUT2 + 1])

        def vr(name, prows=128):
            o = BLOB_R_OFF[name]
            fl = next(s for s in BLOB_R_SPEC if s[0] == name)[2]
            return blob_r[0:prows, o:o + fl]

        def vf(name, prows=128):
            o = BLOB_F_OFF[name]
            fl = next(s for s in BLOB_F_SPEC if s[0] == name)[2]
            return blob_f[0:prows, o:o + fl]

        ct = {}
        ct['w0sel'] = vr('w0sel').rearrange("p (j m) -> p j m", j=32)
        ct['ublk'] = vr('ublk').rearrange("p (j m) -> p j m", j=32)
        for k in ('wl2', 'wl3', 'wl4'):
            ct[k] = vr(k)
        ct['w1x'] = vr('w1x', 1)
        ct['w2x'] = vr('w2x', 1)
        ct['w1f1'] = vr('w1f1', 64)
        ct['w1f2'] = blob_r[:, BLOB_R_OFF['w1f2']:BLOB_R_OFF['w1f2'] + 64]
        ct['wb1'] = vr('wb1', 65)
        ct['wb2'] = vr('wb2', 65)
        ident_r = vr('ident')
        ct['bcol'] = vf('bcol')
        ct['iota'] = vf('iota')
        ct['smask'] = vf('smask')
        c_nat = vr('c_nat').rearrange("p (a k) -> p a k", a=NCH)
        x11t = blob_r[0:1, BLOB_R_OFF['x11t']:BLOB_R_OFF['x11t'] + RC]
        x_row = vf('x_row').rearrange("p (a t) -> p a t", t=2)
        gpat = vf('gpat')

        # ---- stage 1: transpose c -> cT (cols on partitions, rows on free) ----
        cT = p_cdat.tile([128, RC], F32R, tag="cT")
        with tc.tile_pool(name="pstr", bufs=2, space="PSUM") as ps_tr:
            for i in range(NCH):
                pt = ps_tr.tile([128, 128], F32R, tag="misc")
                nc.tensor.transpose(pt[:], c_nat[:, i, :], ident_r[:])
                nc.vector.tensor_copy(cT[:, 128 * i:128 * (i + 1)], pt[:])

        ps_w = ctx.enter_context(tc.tile_pool(name="psw", bufs=4, space="PSUM"))
        es_p12 = ExitStack()
        ps_p12 = es_p12.enter_context(tc.tile_pool(name="psp12", bufs=1, space="PSUM"))

        # ---- stage 2: conditioner MLP, fused first-layer contraction ----
        # four 512-row chains interleaved stage-major: engine streams always
        # have ready work; psum = 4 work slots + 4 accumulators = 8 banks
        NB = 4
        RB = RC // NB            # 512
        h1_1 = p_hs.tile([64, RC], F32R, tag="h1_1")
        p2c = p_hs.tile([128, RC], F32, tag="p2c")
        p12s = [ps_p12.tile([128, RB], F32, tag=f"p12_{b}", name=f"p12_{b}") for b in range(NB)]

        for j in range(32):
            psa = [ps_w.tile([128, RB], F32, tag="w", name=f"psa{j}_{b}") for b in range(NB)]
            for b in range(NB):
                nc.tensor.matmul(psa[b][:], ct['w0sel'][:, j, :],
                                 cT[:, b * RB:(b + 1) * RB], start=True, stop=True)
            a1 = [p_acts.tile([128, RB], F32R, tag="a1", bufs=4, name=f"a1_{j}_{b}") for b in range(NB)]
            for b in range(NB):
                nc.scalar.activation(a1[b][:], psa[b][:], AF.Relu, bias=ct['bcol'][:, 0:1])
            psb = [ps_w.tile([128, RB], F32, tag="w", name=f"psb{j}_{b}") for b in range(NB)]
            for b in range(NB):
                nc.tensor.matmul(psb[b][:], ct['wl2'][:], a1[b][:], start=True, stop=True)
            a2 = [p_acts.tile([128, RB], F32R, tag="a2", bufs=4, name=f"a2_{j}_{b}") for b in range(NB)]
            for b in range(NB):
                nc.vector.tensor_scalar(a2[b][:], psb[b][:], ct['bcol'][:, 1:2], 0.0, ALU.add, ALU.max)
            psc = [ps_w.tile([128, RB], F32, tag="w", name=f"psc{j}_{b}") for b in range(NB)]
            for b in range(NB):
                nc.tensor.matmul(psc[b][:], ct['wl3'][:], a2[b][:], start=True, stop=True)
            m1 = [p_acts.tile([128, RB], F32R, tag="m1", bufs=4, name=f"m1_{j}_{b}") for b in range(NB)]
            for b in range(NB):
                nc.scalar.activation(m1[b][:], psc[b][:], AF.Relu, bias=ct['bcol'][:, 2:3])
            psd = [ps_w.tile([128, RB], F32, tag="w", name=f"psd{j}_{b}") for b in range(NB)]
            for b in range(NB):
                nc.tensor.matmul(psd[b][:], ct['wl4'][:], m1[b][:], start=True, stop=True)
            m2 = [p_acts.tile([128, RB], F32R, tag="m2", bufs=4, name=f"m2_{j}_{b}") for b in range(NB)]
            for b in range(NB):
                nc.vector.tensor_scalar(m2[b][:], psd[b][:], ct['bcol'][:, 3:4], 0.0, ALU.add, ALU.max)
            for b in range(NB):
                nc.tensor.matmul(p12s[b][:], ct['ublk'][:, j, :], m2[b][:],
                                 start=(j == 0), stop=False)
        for b in range(NB):
            nc.tensor.matmul(p12s[b][:], ct['w1x'][:], x11t[:, b * RB:(b + 1) * RB],
                             start=False, stop=True)
        for b in range(NB):
            sl = slice(b * RB, (b + 1) * RB)
            nc.scalar.activation(h1_1[:, sl], p12s[b][0:64, :], AF.Relu, bias=ct['bcol'][0:64, 4:5])
            nc.scalar.activation(p2c[64:128, sl], p12s[b][64:128, :], AF.Identity, bias=ct['bcol'][64:128, 4:5])
        es_p12.close()
        ps_spl = ctx.enter_context(tc.tile_pool(name="psspl", bufs=2, space="PSUM"))

        # ---- stage 3: f1 chain ----
        h2_1 = p_hs.tile([65, RC], F32R, tag="h2_1")
        h2_2 = p_hs.tile([65, RC], F32R, tag="h2_2")
        ones_b = _bc(ct['iota'][64:65, 12:13], 2, RC)[:, 0, :]
        nc.vector.tensor_copy(h2_1[64:65, :], ones_b)
        nc.vector.tensor_copy(h2_2[64:65, :], ones_b)
        for b in range(NB):
            psh = ps_w.tile([64, RB], F32, tag="w", name=f"psh_{b}")
            nc.tensor.matmul(psh[:], ct['w1f1'][:], h1_1[:, b * RB:(b + 1) * RB],
                             start=True, stop=True)
            nc.scalar.activation(h2_1[0:64, b * RB:(b + 1) * RB], psh[:], AF.Relu,
                                 bias=ct['bcol'][0:64, 5:6])

        p1ps = ps_spl.tile([128, NCH, 32], F32, tag="spl")
        for i in range(NCH):
            nc.tensor.matmul(p1ps[:, i, :], h2_1[:, 128 * i:128 * (i + 1)], ct['wb1'][:],
                             start=True, stop=True)

        # ---- spline evaluation (row-major) ----
        def spline(pps, x_in, ytile, pdtile, sidx):
            # pps: psum (128, NCH, 32); x_in: (128, NCH, 1) view
            E = p_spl.tile([128, NCH * 20], F32, tag="E")
            E3 = E[:].rearrange("p (a k) -> p a k", k=20)
            nc.scalar.activation(E3, pps[:, :, 0:20], AF.Exp)
            SpE = p_spl.tile([128, NCH, 9], F32, tag="SpE")
            nc.scalar.activation(SpE[:], pps[:, :, 20:29], AF.Exp)
            SpP = p_spl.tile([128, NCH, 9], F32, tag="SpP")
            nc.vector.tensor_scalar(SpP[:], SpE[:], 1.0, None, ALU.add)
            SpD = p_spl.tile([128, NCH, 9], F32, tag="SpD")
            nc.scalar.activation(SpD[:], SpP[:], AF.Ln)
            CS = p_spl.tile([128, NCH * 20], F32, tag="CS")
            nc.vector.tensor_tensor_scan(CS[:], ct['smask'][:], E[:], 0.0, ALU.mult, ALU.add)
            CS3 = CS[:].rearrange("p (a k) -> p a k", k=20)
            R2 = p_spl.tile([128, NCH, 2], F32, tag="R2")
            nc.vector.reciprocal(R2[:, :, 0:1], CS3[:, :, 9:10])
            nc.vector.reciprocal(R2[:, :, 1:2], CS3[:, :, 19:20])

            GAT = p_spl.tile([128, NCH, 3, 12], F32, tag="GAT")
            nc.vector.tensor_copy(GAT[:].rearrange("p a b c -> p a (b c)"), _bc(gpat, 1, NCH))
            TMPW = p_spl.tile([128, NCH, 10], F32, tag="TMPW")
            nc.vector.tensor_tensor(TMPW[:], CS3[:, :, 0:10], _bc(R2[:, :, 0:1], 2, 10), ALU.mult)
            nc.vector.tensor_scalar(GAT[:, :, 0, 1:11], TMPW[:], 2 * BINT, -BINT, ALU.mult, ALU.add)
            TMPH = p_spl.tile([128, NCH, 10], F32, tag="TMPH")
            nc.vector.tensor_tensor(TMPH[:], CS3[:, :, 10:20], _bc(R2[:, :, 1:2], 2, 10), ALU.mult)
            nc.vector.tensor_scalar(GAT[:, :, 1, 1:11], TMPH[:], 2 * BINT, -BINT, ALU.mult, ALU.add)
            nc.vector.tensor_scalar(GAT[:, :, 2, 1:10], SpD[:], 0.001, None, ALU.add)

            CMP = p_spl.tile([128, NCH, 11], F32, tag="CMP")
            nc.vector.tensor_tensor(CMP[:], GAT[:, :, 0, 0:11], _bc(x_in, 2, 11), ALU.is_le)
            IDX = p_frm.tile([128, NCH], F32, tag=f"IDX{sidx}")
            nc.vector.tensor_reduce(IDX[:], CMP[:], AX.X, ALU.add)
            nc.vector.tensor_scalar(IDX[:], IDX[:], 1.0, 10.0, ALU.max, ALU.min)
            IDXb = _bc(IDX[:], 2, 12)
            OHR = p_spl.tile([128, NCH, 12], F32, tag="OHR")
            iot = ct['iota'][:, 0:12]
            nc.vector.tensor_tensor(OHR[:], _bc(iot, 1, NCH), IDXb, ALU.is_equal)
            OHL = p_spl.tile([128, NCH, 12], F32, tag="OHL")
            iop = ct['iota'][:, 12:24]
            nc.vector.tensor_tensor(OHL[:], _bc(iop, 1, NCH), IDXb, ALU.is_equal)

            TMP = p_spl.tile([128, NCH, 3, 12], F32, tag="TMP")
            PL = p_spl.tile([128, NCH, 3], F32, tag="PL")
            nc.vector.tensor_tensor(TMP[:], GAT[:], _bc(OHL[:], 2, 3), ALU.mult)
            nc.vector.tensor_reduce(PL[:], TMP[:], AX.X, ALU.add)
            PR = p_spl.tile([128, NCH, 3], F32, tag="PR")
            nc.vector.tensor_tensor(TMP[:], GAT[:], _bc(OHR[:], 2, 3), ALU.mult)
            nc.vector.tensor_reduce(PR[:], TMP[:], AX.X, ALU.add)

            def frm(tag):
                t = p_frm.tile([128, NCH], F32, tag=f"{tag}{sidx}")
                return t, t[:].rearrange("p (a k) -> p a k", k=1)
            wl = PL[:, :, 0:1]; hl = PL[:, :, 1:2]; dl = PL[:, :, 2:3]
            wr = PR[:, :, 0:1]; hr = PR[:, :, 1:2]; dr = PR[:, :, 2:3]
            DWH = p_spl.tile([128, NCH, 2], F32, tag="DWH")
            nc.vector.tensor_tensor(DWH[:], PR[:, :, 0:2], PL[:, :, 0:2], ALU.subtract)
            dh = DWH[:, :, 1:2]
            RDW, RDWv = frm("RDW")
            nc.vector.reciprocal(RDWv, DWH[:, :, 0:1])
            T0, T0v = frm("T0")
            nc.vector.tensor_tensor(T0v, x_in, wl, ALU.subtract)
            T, Tv = frm("T")
            nc.vector.tensor_tensor(Tv, T0v, RDWv, ALU.mult)
            S, Sv = frm("S")
            nc.vector.tensor_tensor(Sv, dh, RDWv, ALU.mult)
            T1, T1v = frm("T1")
            nc.vector.tensor_scalar(T1v, Tv, -1.0, 1.0, ALU.mult, ALU.add)
            U_, Uv = frm("U")
            nc.vector.tensor_tensor(Uv, Tv, T1v, ALU.mult)
            T2, T2v = frm("T2")
            nc.scalar.activation(T2v, Tv, AF.Square)
            V1a, V1av = frm("V1a")
            nc.vector.tensor_tensor(V1av, Sv, T2v, ALU.mult)
            V1b, V1bv = frm("V1b")
            nc.vector.tensor_tensor(V1bv, dl, Uv, ALU.mult)
            V1, V1v = frm("V1")
            nc.vector.tensor_tensor(V1v, V1av, V1bv, ALU.add)
            A_, A_v = frm("A_")
            nc.vector.tensor_tensor(A_v, dr, dl, ALU.add)
            B_, B_v = frm("B_")
            nc.vector.scalar_tensor_tensor(B_v, Sv, -2.0, A_v, ALU.mult, ALU.add)
            C_, C_v = frm("C_")
            nc.vector.tensor_tensor(C_v, B_v, Uv, ALU.mult)
            V2, V2v = frm("V2")
            nc.vector.tensor_tensor(V2v, C_v, Sv, ALU.add)
            RV2, RV2v = frm("RV2")
            nc.vector.reciprocal(RV2v, V2v)
            Y1, Y1v = frm("Y1")
            nc.vector.tensor_tensor(Y1v, V1v, RV2v, ALU.mult)
            Y2, Y2v = frm("Y2")
            nc.vector.tensor_tensor(Y2v, Y1v, dh, ALU.mult)
            yv = ytile[:].rearrange("p (a k) -> p a k", k=1)
            nc.vector.tensor_tensor(yv, Y2v, hl, ALU.add)
            N1, N1v = frm("N1")
            nc.vector.tensor_tensor(N1v, dr, T2v, ALU.mult)
            N2, N2v = frm("N2")
            nc.vector.scalar_tensor_tensor(N2v, Sv, 2.0, Uv, ALU.mult, ALU.mult)
            T1S, T1Sv = frm("T1S")
            nc.scalar.activation(T1Sv, T1v, AF.Square)
            N3, N3v = frm("N3")
            nc.vector.tensor_tensor(N3v, dl, T1Sv, ALU.mult)
            N12, N12v = frm("N12")
            nc.vector.tensor_tensor(N12v, N1v, N2v, ALU.add)
            NUM, NUMv = frm("NUM")
            nc.vector.tensor_tensor(NUMv, N12v, N3v, ALU.add)
            S2, S2v = frm("S2")
            nc.scalar.activation(S2v, Sv, AF.Square)
            RVQ, RVQv = frm("RVQ")
            nc.scalar.activation(RVQv, RV2v, AF.Square)
            PDa, PDav = frm("PDa")
            nc.vector.tensor_tensor(PDav, S2v, NUMv, ALU.mult)
            pdv = pdtile[:].rearrange("p (a k) -> p a k", k=1)
            nc.vector.tensor_tensor(pdv, PDav, RVQv, ALU.mult)

        x22row = p_out.tile([128, NCH], F32, tag="x22row")
        pd1 = p_out.tile([128, NCH], F32, tag="pd1")
        spline(p1ps, x_row[:, :, 1:2], x22row, pd1, 1)

        # ---- x22 transpose -> x22t (1, RC) fp32r ----
        x22r = p_out.tile([128, NCH], F32R, tag="x22r")
        nc.vector.tensor_copy(x22r[:], x22row[:])
        ptx = ps_spl.tile([16, 128], F32R, tag="spl")
        nc.tensor.transpose(ptx[0:NCH, :], x22r[:, :], ident_r[:])
        xt16 = p_cdat.tile([16, 128], F32R, tag="xt16")
        nc.vector.tensor_copy(xt16[:], ptx[0:NCH, :])
        x22t = p_cdat.tile([1, RC], F32R, tag="x22t")
        nc.gpsimd.dma_start(x22t[:], xt16[:])

        # ---- f2 chain ----
        h1_2 = p_hs.tile([128, RC], F32R, tag="h1_2")
        for b in range(NB):
            psx = ps_w.tile([128, RB], F32, tag="w")
            nc.tensor.matmul(psx[:], ct['w2x'][:], x22t[:, b * RB:(b + 1) * RB],
                             start=True, stop=True)
            sl = slice(b * RB, (b + 1) * RB)
            TS = p_hs.tile([128, RB], F32, tag="TS")
            nc.vector.tensor_tensor(TS[64:128, :], psx[64:128, :], p2c[64:128, sl], ALU.add)
            nc.vector.tensor_scalar(h1_2[64:128, sl], TS[64:128, :], 0.0, None, ALU.max)
        for b in range(NB):
            psh2 = ps_w.tile([64, RB], F32, tag="w")
            nc.tensor.matmul(psh2[:], ct['w1f2'][64:128, :],
                             h1_2[64:128, b * RB:(b + 1) * RB], start=True, stop=True)
            nc.vector.tensor_scalar(h2_2[0:64, b * RB:(b + 1) * RB], psh2[:],
                                    ct['bcol'][0:64, 6:7], 0.0, ALU.add, ALU.max)

        p2ps = ps_spl.tile([128, NCH, 32], F32, tag="spl")
        for i in range(NCH):
            nc.tensor.matmul(p2ps[:, i, :], h2_2[:, 128 * i:128 * (i + 1)], ct['wb2'][:],
                             start=True, stop=True)

        x31row = p_out.tile([128, NCH], F32, tag="x31row")
        pd2 = p_out.tile([128, NCH], F32, tag="pd2")
        spline(p2ps, x_row[:, :, 0:1], x31row, pd2, 2)

        # ---- outputs ----
        x3row = p_out.tile([128, NCH, 2], F32, tag="x3row")
        nc.vector.tensor_copy(x3row[:, :, 0], x31row[:])
        nc.vector.tensor_copy(x3row[:, :, 1], x22row[:])
        jam = p_out.tile([128, NCH], F32, tag="jam")
        nc.vector.tensor_tensor(jam[:], pd1[:], pd2[:], ALU.mult)
        jarow = p_out.tile([128, NCH], F32, tag="jarow")
        nc.scalar.activation(jarow[:], jam[:], AF.Abs)
        nc.gpsimd.dma_start(d_ox3[:, :, :], x3row[:])
        nc.gpsimd.dma_start(d_oja[:, :], jarow[:])

    nc.compile()
    return nc


# ---------------------------------------------------------------------------
# entry point
# ---------------------------------------------------------------------------
_CACHE = {}


def kernel(**inputs):
    from concourse.bass_utils import run_bass_kernel_spmd

    consts = build_consts(inputs)
    x = np.ascontiguousarray(np.asarray(inputs['x'], np.float32))
    c = np.ascontiguousarray(np.asarray(inputs['c'], np.float32))

    if 'nc' not in _CACHE:
        _CACHE['nc'] = build_program()
    nc = _CACHE['nc']

    in_maps = []
    for m in range(NCORES):
        xs = x[m * RC:(m + 1) * RC]
        cs = c[m * RC:(m + 1) * RC]
        br, bf = pack_blobs(consts, cs, xs)
        in_maps.append({'blob_r': br, 'blob_f': bf})

    res = run_bass_kernel_spmd(nc, in_maps, core_ids=list(range(NCORES)))
    x3 = np.empty((B, 2), np.float32)
    ja = np.empty((B, 1), np.float32)
    for m in range(NCORES):
        r = res.results[m]
        x3[m * RC:(m + 1) * RC] = r['o_x3'].transpose(1, 0, 2).reshape(RC, 2)
        ja[m * RC:(m + 1) * RC] = r['o_ja'].transpose(1, 0).reshape(RC, 1)
    return x3, ja


if __name__ == "__main__":
    nc = build_program()
    print("program built ok")


# revision 22
# speedup vs baseline: 1.6283x; 1.0107x over previous
"""CSplineBasic Trainium2 kernel: conditional rational-quadratic spline flow.

Strategy (pure data-parallel over batch, 8 cores):
  - Host precomputes weight-only transforms: the fa/fm conditioner MLPs are
    fused (fa_W2@fm_W0), the final fm_W2 projection and the f1/f2 first-layer
    contraction over the 129 conditioner outputs are factorized into
    per-column-slab matrices U_j[(g,f),:] = fm_W2[f]*V12[4j+g,:], with the
    null-token masking and the constant v-column folded into biases.
  - Device evaluates the scalar conditioner g(c) for all B*128 elements as
    fp32r matmuls (4 col-groups x 32 features on partitions, rows on free),
    accumulating the f1/f2 first-layer pre-activations directly in PSUM.
  - f1 -> spline1 (row-major searchsorted via compare+reduce, one-hot
    gathers, rational-quadratic formula) -> f2 -> spline2 -> outputs.
"""
import sys
for _p in ('/opt/trn_rl_repo', '/root/.axon_site/_ro/trn_rl_repo'):
    if _p not in sys.path:
        sys.path.append(_p)

from contextlib import ExitStack
import numpy as np

import concourse.bass as bass
import concourse.tile as tile
from concourse import bacc
import concourse.mybir as mybir

F32 = mybir.dt.float32
F32R = mybir.dt.float32r
AF = mybir.ActivationFunctionType
ALU = mybir.AluOpType
AX = mybir.AxisListType

B = 16384
NCORES = 8
RC = B // NCORES          # rows per core (2048)
NCH = RC // 128           # row chunks (16)
COND = 128
KNOT = 10
BINT = 5.0
HID = 64


# ---------------------------------------------------------------------------
# host-side constant building (weights only)
# ---------------------------------------------------------------------------
def build_consts(inp):
    f64 = np.float64
    faW = [np.asarray(inp[f'fa_W{i}'], f64) for i in range(3)]
    fab = [np.asarray(inp[f'fa_b{i}'], f64) for i in range(3)]
    fmW = [np.asarray(inp[f'fm_W{i}'], f64) for i in range(3)]
    fmb = [np.asarray(inp[f'fm_b{i}'], f64) for i in range(3)]
    f1W = [np.asarray(inp[f'f1_W{i}'], f64) for i in range(3)]
    f1b = [np.asarray(inp[f'f1_b{i}'], f64) for i in range(3)]
    f2W = [np.asarray(inp[f'f2_W{i}'], f64) for i in range(3)]
    f2b = [np.asarray(inp[f'f2_b{i}'], f64) for i in range(3)]
    ip = int(inp['index_p'])
    iv = int(inp['index_v'])

    def ffn(z, Ws, bs):
        for i in range(len(Ws)):
            z = z @ Ws[i] + bs[i]
            if i < len(Ws) - 1:
                z = np.maximum(z, 0)
        return z

    Wf = faW[2] @ fmW[0]
    bf = fab[2] @ fmW[0] + fmb[0]
    kappa = ffn(np.asarray(inp['null_token'], f64), fmW, fmb)[0, 0]
    nu = ffn(ffn(np.array([[np.sin(f64(iv))]]), faW, fab), fmW, fmb)[0, 0]

    V12 = np.concatenate([f1W[0][1:COND + 1], f2W[0][1:COND + 1]], axis=1)  # (128,128)
    mask = np.ones(COND, bool)
    mask[ip] = False
    mask[ip + COND // 2] = False

    c = {}
    # L1 selector weights: w0sel[j, p, 32g+f] = fa_W0[0,f] * [p == 4j+g]
    w0sel = np.zeros((32, 128, 128), np.float32)
    for j in range(32):
        for g in range(4):
            w0sel[j, 4 * j + g, 32 * g:32 * g + 32] = faW[0][0].astype(np.float32)
    c['w0sel'] = np.ascontiguousarray(w0sel.transpose(1, 0, 2))
    # U matrices: ublk[j, 32g+f, m] = fm_W2[f,0] * V12[4j+g, m]  (masked cols zero)
    ublk = np.zeros((32, 128, 128), np.float32)
    for j in range(32):
        for g in range(4):
            cc = 4 * j + g
            if mask[cc]:
                ublk[j, 32 * g:32 * g + 32, :] = np.outer(fmW[2][:, 0], V12[cc]).astype(np.float32)
    c['ublk'] = np.ascontiguousarray(ublk.transpose(1, 0, 2))

    def blockdiag4(W):
        out = np.zeros((128, 128), np.float32)
        for g in range(4):
            out[32 * g:32 * g + 32, 32 * g:32 * g + 32] = W.astype(np.float32)
        return out
    c['wl2'] = blockdiag4(faW[1])
    c['wl3'] = blockdiag4(Wf)
    c['wl4'] = blockdiag4(fmW[1])

    Vsum = V12[mask].sum(0)
    b0eff = np.concatenate([
        nu * f1W[0][COND + 1] + f1b[0] + kappa * (V12[ip, :64] + V12[ip + 64, :64]),
        nu * f2W[0][COND + 1] + f2b[0] + kappa * (V12[ip, 64:] + V12[ip + 64, 64:]),
    ]) + fmb[2][0] * Vsum

    bcol = np.zeros((128, 8), np.float32)
    bcol[:, 0] = np.tile(fab[0], 4)
    bcol[:, 1] = np.tile(fab[1], 4)
    bcol[:, 2] = np.tile(bf, 4)
    bcol[:, 3] = np.tile(fmb[1], 4)
    bcol[:, 4] = b0eff.astype(np.float32)
    bcol[0:64, 5] = f1b[1].astype(np.float32)
    bcol[0:64, 6] = f2b[1].astype(np.float32)
    c['bcol'] = bcol

    w1x = np.zeros((1, 128), np.float32)
    w1x[0, 0:64] = f1W[0][0].astype(np.float32)
    c['w1x'] = w1x
    w2x = np.zeros((1, 128), np.float32)
    w2x[0, 64:128] = f2W[0][0].astype(np.float32)
    c['w2x'] = w2x

    c['w1f1'] = f1W[1].astype(np.float32)                     # (64,64)
    w1f2 = np.zeros((128, 64), np.float32)
    w1f2[64:128] = f2W[1].astype(np.float32)
    c['w1f2'] = w1f2

    def wbmat(W2, b2):
        out = np.zeros((65, 32), np.float32)
        out[0:64, 0:29] = W2.astype(np.float32)
        out[64, 0:29] = b2.astype(np.float32)
        return out
    c['wb1'] = wbmat(f1W[2], f1b[2])
    c['wb2'] = wbmat(f2W[2], f2b[2])

    c['ident'] = np.eye(128, dtype=np.float32)
    iota = np.zeros((128, 24), np.float32)
    iota[:, 0:12] = np.arange(12)
    iota[:, 12:24] = np.arange(12) + 1
    c['iota'] = iota
    sm = np.ones(320, np.float32)
    sm[::10] = 0.0
    c['smask'] = np.tile(sm[None, :], (128, 1))
    gp = np.zeros(36, np.float32)
    gp[0] = -BINT          # w slot 0
    gp[12] = -BINT         # h slot 0
    gp[24] = 1.0           # d slot 0
    gp[34] = 1.0           # d slot 10
    c['gpat'] = np.tile(gp[None, :], (128, 1))
    return c


CONST_DTYPES = {
    'w0sel': F32R, 'ublk': F32R, 'wl2': F32R, 'wl3': F32R, 'wl4': F32R,
    'w1x': F32R, 'w2x': F32R, 'w1f1': F32R, 'w1f2': F32R,
    'wb1': F32R, 'wb2': F32R,
    'bcol': F32, 'iota': F32, 'smask': F32,
}




# blob packing: (name, partition_rows, free_elems) in order
BLOB_R_SPEC = [
    ('c_nat', 128, 16 * 128), ('x11t', 1, 2048), ('ident', 128, 128),
    ('w0sel', 128, 32 * 128),
    ('wl2', 128, 128), ('wl3', 128, 128), ('wl4', 128, 128),
    ('w1x', 1, 128), ('w2x', 1, 128), ('w1f1', 64, 64), ('w1f2', 128, 64),
    ('wb1', 65, 32), ('wb2', 65, 32),
    ('ublk', 128, 32 * 128),
]
BLOB_F_SPEC = [
    ('bcol', 128, 8), ('iota', 128, 24), ('smask', 128, 320),
    ('x_row', 128, 32), ('gpat', 128, 36),
]
BLOB_R_OFF = {}
_o = 0
for _n, _p, _f in BLOB_R_SPEC:
    BLOB_R_OFF[_n] = _o; _o += _f
BLOB_R_LEN = _o
BLOB_F_OFF = {}
_o = 0
for _n, _p, _f in BLOB_F_SPEC:
    BLOB_F_OFF[_n] = _o; _o += _f
BLOB_F_LEN = _o


def pack_blobs(consts, c_shard, x_shard):
    br = np.zeros((128, BLOB_R_LEN), np.float32)
    def put_r(name, arr):
        o = BLOB_R_OFF[name]
        arr = np.asarray(arr, np.float32)
        p = arr.shape[0]
        br[:p, o:o + int(np.prod(arr.shape[1:]))] = arr.reshape(p, -1)
    for n in ('w0sel', 'ublk', 'wl2', 'wl3', 'wl4', 'w1x', 'w2x', 'w1f1',
              'wb1', 'wb2', 'ident'):
        put_r(n, consts[n])
    br[64:128, BLOB_R_OFF['w1f2']:BLOB_R_OFF['w1f2'] + 64] = consts['w1f2'][64:128]
    cn = c_shard.reshape(16, 128, 128).transpose(1, 0, 2).reshape(128, 2048)
    br[:, BLOB_R_OFF['c_nat']:BLOB_R_OFF['c_nat'] + 2048] = cn
    br[0, BLOB_R_OFF['x11t']:BLOB_R_OFF['x11t'] + 2048] = x_shard[:, 0]
    bf = np.zeros((128, BLOB_F_LEN), np.float32)
    def put_f(name, arr):
        o = BLOB_F_OFF[name]
        arr = np.asarray(arr, np.float32)
        p = arr.shape[0]
        bf[:p, o:o + int(np.prod(arr.shape[1:]))] = arr.reshape(p, -1)
    put_f('bcol', consts['bcol'])
    put_f('iota', consts['iota'])
    put_f('smask', consts['smask'])
    put_f('x_row', x_shard.reshape(16, 128, 2).transpose(1, 0, 2).reshape(128, 32))
    put_f('gpat', consts['gpat'])
    return br, bf


def _bc(ap, pos, n):
    """insert a step-0 broadcast dim of size n at position pos of an AP"""
    lst = [list(d) for d in ap.ap]
    lst.insert(pos, [0, n])
    return bass.AP(tensor=ap.tensor, offset=ap.offset, ap=lst)


# ---------------------------------------------------------------------------
# the bass program
# ---------------------------------------------------------------------------
def build_program():
    nc = bacc.Bacc("TRN2", target_bir_lowering=False)

    d_br = nc.dram_tensor("blob_r", [128, BLOB_R_LEN], F32, kind="ExternalInput")
    d_bf = nc.dram_tensor("blob_f", [128, BLOB_F_LEN], F32, kind="ExternalInput")
    d_ox3 = nc.dram_tensor("o_x3", [128, NCH, 2], F32, kind="ExternalOutput")
    d_oja = nc.dram_tensor("o_ja", [128, NCH], F32, kind="ExternalOutput")

    with ExitStack() as ctx:
        tc = ctx.enter_context(tile.TileContext(nc))
        p_const = ctx.enter_context(tc.tile_pool(name="const", bufs=1))
        p_cdat = ctx.enter_context(tc.tile_pool(name="cdat", bufs=1))
        p_acts = ctx.enter_context(tc.tile_pool(name="acts", bufs=2))
        p_hs = ctx.enter_context(tc.tile_pool(name="hs", bufs=1))
        p_spl = ctx.enter_context(tc.tile_pool(name="spl", bufs=2))
        p_frm = ctx.enter_context(tc.tile_pool(name="frm", bufs=2))
        p_out = ctx.enter_context(tc.tile_pool(name="out", bufs=1))

        # ---- load blobs (2 DMAs; fp32r blob cast on the way in) ----
        blob_r = p_const.tile([128, BLOB_R_LEN], F32R, tag="blob_r")
        CUT1 = BLOB_R_OFF['w0sel']          # end of data+ident section
        CUT2 = BLOB_R_OFF['ublk']           # start of ublk section
        nc.gpsimd.dma_start(blob_r[:, 0:CUT1], d_br[:, 0:CUT1])
        nc.gpsimd.dma_start(blob_r[:, CUT1:CUT2], d_br[:, CUT1:CUT2])
        nc.gpsimd.dma_start(blob_r[:, CUT2:], d_br[:, CUT2:])
        blob_f = p_const.tile([128, BLOB_F_LEN], F32, tag="blob_f")
        nc.gpsimd.dma_start(blob_f[:], d_bf[:, :])

        # tiny observer ops: absorb the two DMA-lane sem ticks into each
        # engine's vector clock once, so no real instruction needs >1 wait
        dob = p_const.tile([1, 8], F32, tag="dob")
        dobr = p_const.tile([1, 8], F32R, tag="dobr")
        nc.scalar.copy(dob[0:1, 0:1], blob_f[0:1, 0:1])
        nc.vector.tensor_copy(dob[0:1, 1:2], blob_f[0:1, 0:1])
        nc.vector.tensor_copy(dobr[0:1, 1:2], blob_r[0:1, 0:1])
        nc.scalar.copy(dobr[0:1, 2:3], blob_r[0:1, 0:1])
        nc.vector.tensor_copy(dobr[0:1, 3:4], blob_r[0:1, CUT1:CUT1 + 1])
        nc.scalar.copy(dobr[0:1, 4:5], blob_r[0:1, CUT1:CUT1 + 1])
        nc.vector.tensor_copy(dobr[0:1, 5:6], blob_r[0:1, CUT2:CUT2 + 1])
        nc.scalar.copy(dobr[0:1, 6:7], blob_r[0:1, CUT2:CUT2 + 1])

        def vr(name, prows=128):
            o = BLOB_R_OFF[name]
            fl = next(s for s in BLOB_R_SPEC if s[0] == name)[2]
            return blob_r[0:prows, o:o + fl]

        def vf(name, prows=128):
            o = BLOB_F_OFF[name]
            fl = next(s for s in BLOB_F_SPEC if s[0] == name)[2]
            return blob_f[0:prows, o:o + fl]

        ct = {}
        ct['w0sel'] = vr('w0sel').rearrange("p (j m) -> p j m", j=32)
        ct['ublk'] = vr('ublk').rearrange("p (j m) -> p j m", j=32)
        for k in ('wl2', 'wl3', 'wl4'):
            ct[k] = vr(k)
        ct['w1x'] = vr('w1x', 1)
        ct['w2x'] = vr('w2x', 1)
        ct['w1f1'] = vr('w1f1', 64)
        ct['w1f2'] = blob_r[:, BLOB_R_OFF['w1f2']:BLOB_R_OFF['w1f2'] + 64]
        ct['wb1'] = vr('wb1', 65)
        ct['wb2'] = vr('wb2', 65)
        ident_r = vr('ident')
        ct['bcol'] = vf('bcol')
        ct['iota'] = vf('iota')
        ct['smask'] = vf('smask')
        c_nat = vr('c_nat').rearrange("p (a k) -> p a k", a=NCH)
        x11t = blob_r[0:1, BLOB_R_OFF['x11t']:BLOB_R_OFF['x11t'] + RC]
        x_row = vf('x_row').rearrange("p (a t) -> p a t", t=2)
        gpat = vf('gpat')

        # ---- stage 1: transpose c -> cT (cols on partitions, rows on free) ----
        cT = p_cdat.tile([128, RC], F32R, tag="cT")
        with tc.tile_pool(name="pstr", bufs=2, space="PSUM") as ps_tr:
            for i in range(NCH):
                pt = ps_tr.tile([128, 128], F32R, tag="misc")
                nc.tensor.transpose(pt[:], c_nat[:, i, :], ident_r[:])
                if i % 2 == 0:
                    nc.vector.tensor_copy(cT[:, 128 * i:128 * (i + 1)], pt[:])
                else:
                    nc.scalar.copy(cT[:, 128 * i:128 * (i + 1)], pt[:])

        ps_w = ctx.enter_context(tc.tile_pool(name="psw", bufs=4, space="PSUM"))
        es_p12 = ExitStack()
        ps_p12 = es_p12.enter_context(tc.tile_pool(name="psp12", bufs=1, space="PSUM"))

        # ---- stage 2: conditioner MLP, fused first-layer contraction ----
        # four 512-row chains interleaved stage-major: engine streams always
        # have ready work; psum = 4 work slots + 4 accumulators = 8 banks
        NB = 4
        RB = RC // NB            # 512
        h1_1 = p_hs.tile([64, RC], F32R, tag="h1_1")
        p2c = p_hs.tile([128, RC], F32, tag="p2c")
        p12s = [ps_p12.tile([128, RB], F32, tag=f"p12_{b}", name=f"p12_{b}") for b in range(NB)]

        for j in range(32):
            psa = [ps_w.tile([128, RB], F32, tag="w", name=f"psa{j}_{b}") for b in range(NB)]
            for b in range(NB):
                nc.tensor.matmul(psa[b][:], ct['w0sel'][:, j, :],
                                 cT[:, b * RB:(b + 1) * RB], start=True, stop=True)
            a1 = [p_acts.tile([128, RB], F32R, tag="a1", bufs=4, name=f"a1_{j}_{b}") for b in range(NB)]
            for b in range(NB):
                nc.scalar.activation(a1[b][:], psa[b][:], AF.Relu, bias=ct['bcol'][:, 0:1])
            psb = [ps_w.tile([128, RB], F32, tag="w", name=f"psb{j}_{b}") for b in range(NB)]
            for b in range(NB):
                nc.tensor.matmul(psb[b][:], ct['wl2'][:], a1[b][:], start=True, stop=True)
            a2 = [p_acts.tile([128, RB], F32R, tag="a2", bufs=4, name=f"a2_{j}_{b}") for b in range(NB)]
            for b in range(NB):
                nc.vector.tensor_scalar(a2[b][:], psb[b][:], ct['bcol'][:, 1:2], 0.0, ALU.add, ALU.max)
            psc = [ps_w.tile([128, RB], F32, tag="w", name=f"psc{j}_{b}") for b in range(NB)]
            for b in range(NB):
                nc.tensor.matmul(psc[b][:], ct['wl3'][:], a2[b][:], start=True, stop=True)
            m1 = [p_acts.tile([128, RB], F32R, tag="m1", bufs=4, name=f"m1_{j}_{b}") for b in range(NB)]
            for b in range(NB):
                nc.scalar.activation(m1[b][:], psc[b][:], AF.Relu, bias=ct['bcol'][:, 2:3])
            psd = [ps_w.tile([128, RB], F32, tag="w", name=f"psd{j}_{b}") for b in range(NB)]
            for b in range(NB):
                nc.tensor.matmul(psd[b][:], ct['wl4'][:], m1[b][:], start=True, stop=True)
            m2 = [p_acts.tile([128, RB], F32R, tag="m2", bufs=4, name=f"m2_{j}_{b}") for b in range(NB)]
            for b in range(NB):
                if (j * NB + b) % 8 != 1:
                    nc.vector.tensor_scalar(m2[b][:], psd[b][:], ct['bcol'][:, 3:4], 0.0, ALU.add, ALU.max)
                else:
                    nc.scalar.activation(m2[b][:], psd[b][:], AF.Relu, bias=ct['bcol'][:, 3:4])
            for b in range(NB):
                nc.tensor.matmul(p12s[b][:], ct['ublk'][:, j, :], m2[b][:],
                                 start=(j == 0), stop=False)
        for b in range(NB):
            nc.tensor.matmul(p12s[b][:], ct['w1x'][:], x11t[:, b * RB:(b + 1) * RB],
                             start=False, stop=True)
        for b in range(NB):
            sl = slice(b * RB, (b + 1) * RB)
            nc.scalar.activation(h1_1[:, sl], p12s[b][0:64, :], AF.Relu, bias=ct['bcol'][0:64, 4:5])
            nc.scalar.activation(p2c[64:128, sl], p12s[b][64:128, :], AF.Identity, bias=ct['bcol'][64:128, 4:5])
        es_p12.close()
        ps_spl = ctx.enter_context(tc.tile_pool(name="psspl", bufs=2, space="PSUM"))

        # ---- stage 3: f1 chain ----
        h2_1 = p_hs.tile([65, RC], F32R, tag="h2_1")
        h2_2 = p_hs.tile([65, RC], F32R, tag="h2_2")
        ones_b = _bc(ct['iota'][64:65, 12:13], 2, RC)[:, 0, :]
        nc.vector.tensor_copy(h2_1[64:65, :], ones_b)
        nc.vector.tensor_copy(h2_2[64:65, :], ones_b)
        for b in range(NB):
            psh = ps_w.tile([64, RB], F32, tag="w", name=f"psh_{b}")
            nc.tensor.matmul(psh[:], ct['w1f1'][:], h1_1[:, b * RB:(b + 1) * RB],
                             start=True, stop=True)
            nc.scalar.activation(h2_1[0:64, b * RB:(b + 1) * RB], psh[:], AF.Relu,
                                 bias=ct['bcol'][0:64, 5:6])

        p1ps = ps_spl.tile([128, NCH, 32], F32, tag="spl")
        for i in range(NCH):
            nc.tensor.matmul(p1ps[:, i, :], h2_1[:, 128 * i:128 * (i + 1)], ct['wb1'][:],
                             start=True, stop=True)

        # ---- spline evaluation (row-major) ----
        def spline(pps, x_in, ytile, pdtile, sidx):
            # pps: psum (128, NCH, 32); x_in: (128, NCH, 1) view
            E = p_spl.tile([128, NCH * 20], F32, tag="E")
            E3 = E[:].rearrange("p (a k) -> p a k", k=20)
            nc.scalar.activation(E3, pps[:, :, 0:20], AF.Exp)
            SpE = p_spl.tile([128, NCH, 9], F32, tag="SpE")
            nc.scalar.activation(SpE[:], pps[:, :, 20:29], AF.Exp)
            SpP = p_spl.tile([128, NCH, 9], F32, tag="SpP")
            nc.vector.tensor_scalar(SpP[:], SpE[:], 1.0, None, ALU.add)
            SpD = p_spl.tile([128, NCH, 9], F32, tag="SpD")
            nc.scalar.activation(SpD[:], SpP[:], AF.Ln)
            CS = p_spl.tile([128, NCH * 20], F32, tag="CS")
            nc.vector.tensor_tensor_scan(CS[:], ct['smask'][:], E[:], 0.0, ALU.mult, ALU.add)
            CS3 = CS[:].rearrange("p (a k) -> p a k", k=20)
            R2 = p_spl.tile([128, NCH, 2], F32, tag="R2")
            nc.vector.reciprocal(R2[:, :, 0:1], CS3[:, :, 9:10])
            nc.vector.reciprocal(R2[:, :, 1:2], CS3[:, :, 19:20])

            GAT = p_spl.tile([128, NCH, 3, 12], F32, tag="GAT")
            nc.gpsimd.tensor_copy(GAT[:].rearrange("p a b c -> p a (b c)"), _bc(gpat, 1, NCH))
            TMPW = p_spl.tile([128, NCH, 10], F32, tag="TMPW")
            nc.vector.tensor_tensor(TMPW[:], CS3[:, :, 0:10], _bc(R2[:, :, 0:1], 2, 10), ALU.mult)
            nc.vector.tensor_scalar(GAT[:, :, 0, 1:11], TMPW[:], 2 * BINT, -BINT, ALU.mult, ALU.add)
            TMPH = p_spl.tile([128, NCH, 10], F32, tag="TMPH")
            nc.vector.tensor_tensor(TMPH[:], CS3[:, :, 10:20], _bc(R2[:, :, 1:2], 2, 10), ALU.mult)
            nc.vector.tensor_scalar(GAT[:, :, 1, 1:11], TMPH[:], 2 * BINT, -BINT, ALU.mult, ALU.add)
            nc.vector.tensor_scalar(GAT[:, :, 2, 1:10], SpD[:], 0.001, None, ALU.add)

            CMP = p_spl.tile([128, NCH, 11], F32, tag="CMP")
            nc.vector.tensor_tensor(CMP[:], GAT[:, :, 0, 0:11], _bc(x_in, 2, 11), ALU.is_le)
            IDX = p_frm.tile([128, NCH], F32, tag=f"IDX{sidx}")
            nc.vector.tensor_reduce(IDX[:], CMP[:], AX.X, ALU.add)
            nc.vector.tensor_scalar(IDX[:], IDX[:], 1.0, 10.0, ALU.max, ALU.min)
            IDXb = _bc(IDX[:], 2, 12)
            OHR = p_spl.tile([128, NCH, 12], F32, tag="OHR")
            iot = ct['iota'][:, 0:12]
            nc.vector.tensor_tensor(OHR[:], _bc(iot, 1, NCH), IDXb, ALU.is_equal)
            OHL = p_spl.tile([128, NCH, 12], F32, tag="OHL")
            iop = ct['iota'][:, 12:24]
            nc.vector.tensor_tensor(OHL[:], _bc(iop, 1, NCH), IDXb, ALU.is_equal)

            TMP = p_spl.tile([128, NCH, 3, 12], F32, tag="TMP")
            PL = p_spl.tile([128, NCH, 3], F32, tag="PL")
            nc.vector.tensor_tensor(TMP[:], GAT[:], _bc(OHL[:], 2, 3), ALU.mult)
            nc.vector.tensor_reduce(PL[:], TMP[:], AX.X, ALU.add)
            PR = p_spl.tile([128, NCH, 3], F32, tag="PR")
            nc.vector.tensor_tensor(TMP[:], GAT[:], _bc(OHR[:], 2, 3), ALU.mult)
            nc.vector.tensor_reduce(PR[:], TMP[:], AX.X, ALU.add)

            def frm(tag):
                t = p_frm.tile([128, NCH], F32, tag=f"{tag}{sidx}")
                return t, t[:].rearrange("p (a k) -> p a k", k=1)
            wl = PL[:, :, 0:1]; hl = PL[:, :, 1:2]; dl = PL[:, :, 2:3]
            wr = PR[:, :, 0:1]; hr = PR[:, :, 1:2]; dr = PR[:, :, 2:3]
            DWH = p_spl.tile([128, NCH, 2], F32, tag="DWH")
            nc.vector.tensor_tensor(DWH[:], PR[:, :, 0:2], PL[:, :, 0:2], ALU.subtract)
            dh = DWH[:, :, 1:2]
            RDW, RDWv = frm("RDW")
            nc.vector.reciprocal(RDWv, DWH[:, :, 0:1])
            T0, T0v = frm("T0")
            nc.vector.tensor_tensor(T0v, x_in, wl, ALU.subtract)
            T, Tv = frm("T")
            nc.vector.tensor_tensor(Tv, T0v, RDWv, ALU.mult)
            S, Sv = frm("S")
            nc.vector.tensor_tensor(Sv, dh, RDWv, ALU.mult)
            T1, T1v = frm("T1")
            nc.vector.tensor_scalar(T1v, Tv, -1.0, 1.0, ALU.mult, ALU.add)
            U_, Uv = frm("U")
            nc.vector.tensor_tensor(Uv, Tv, T1v, ALU.mult)
            T2, T2v = frm("T2")
            nc.scalar.activation(T2v, Tv, AF.Square)
            V1a, V1av = frm("V1a")
            nc.vector.tensor_tensor(V1av, Sv, T2v, ALU.mult)
            V1b, V1bv = frm("V1b")
            nc.vector.tensor_tensor(V1bv, dl, Uv, ALU.mult)
            V1, V1v = frm("V1")
            nc.vector.tensor_tensor(V1v, V1av, V1bv, ALU.add)
            A_, A_v = frm("A_")
            nc.vector.tensor_tensor(A_v, dr, dl, ALU.add)
            B_, B_v = frm("B_")
            nc.vector.scalar_tensor_tensor(B_v, Sv, -2.0, A_v, ALU.mult, ALU.add)
            C_, C_v = frm("C_")
            nc.vector.tensor_tensor(C_v, B_v, Uv, ALU.mult)
            V2, V2v = frm("V2")
            nc.vector.tensor_tensor(V2v, C_v, Sv, ALU.add)
            RV2, RV2v = frm("RV2")
            nc.vector.reciprocal(RV2v, V2v)
            Y1, Y1v = frm("Y1")
            nc.vector.tensor_tensor(Y1v, V1v, RV2v, ALU.mult)
            Y2, Y2v = frm("Y2")
            nc.vector.tensor_tensor(Y2v, Y1v, dh, ALU.mult)
            yv = ytile[:].rearrange("p (a k) -> p a k", k=1)
            nc.vector.tensor_tensor(yv, Y2v, hl, ALU.add)
            N1, N1v = frm("N1")
            nc.vector.tensor_tensor(N1v, dr, T2v, ALU.mult)
            N2, N2v = frm("N2")
            nc.vector.scalar_tensor_tensor(N2v, Sv, 2.0, Uv, ALU.mult, ALU.mult)
            T1S, T1Sv = frm("T1S")
            nc.scalar.activation(T1Sv, T1v, AF.Square)
            N3, N3v = frm("N3")
            nc.vector.tensor_tensor(N3v, dl, T1Sv, ALU.mult)
            N12, N12v = frm("N12")
            nc.vector.tensor_tensor(N12v, N1v, N2v, ALU.add)
            NUM, NUMv = frm("NUM")
            nc.vector.tensor_tensor(NUMv, N12v, N3v, ALU.add)
            S2, S2v = frm("S2")
            nc.scalar.activation(S2v, Sv, AF.Square)
            RVQ, RVQv = frm("RVQ")
            nc.scalar.activation(RVQv, RV2v, AF.Square)
            PDa, PDav = frm("PDa")
            nc.vector.tensor_tensor(PDav, S2v, NUMv, ALU.mult)
            pdv = pdtile[:].rearrange("p (a k) -> p a k", k=1)
            nc.vector.tensor_tensor(pdv, PDav, RVQv, ALU.mult)

        x22row = p_out.tile([128, NCH], F32, tag="x22row")
        pd1 = p_out.tile([128, NCH], F32, tag="pd1")
        spline(p1ps, x_row[:, :, 1:2], x22row, pd1, 1)

        # ---- x22 transpose -> x22t (1, RC) fp32r ----
        x22r = p_out.tile([128, NCH], F32R, tag="x22r")
        nc.vector.tensor_copy(x22r[:], x22row[:])
        ptx = ps_spl.tile([16, 128], F32R, tag="spl")
        nc.tensor.transpose(ptx[0:NCH, :], x22r[:, :], ident_r[:])
        xt16 = p_cdat.tile([16, 128], F32R, tag="xt16")
        nc.vector.tensor_copy(xt16[:], ptx[0:NCH, :])
        x22t = p_cdat.tile([1, RC], F32R, tag="x22t")
        nc.gpsimd.dma_start(x22t[:], xt16[:])

        # ---- f2 chain ----
        h1_2 = p_hs.tile([128, RC], F32R, tag="h1_2")
        for b in range(NB):
            psx = ps_w.tile([128, RB], F32, tag="w")
            nc.tensor.matmul(psx[:], ct['w2x'][:], x22t[:, b * RB:(b + 1) * RB],
                             start=True, stop=True)
            sl = slice(b * RB, (b + 1) * RB)
            TS = p_hs.tile([128, RB], F32, tag="TS")
            nc.vector.tensor_tensor(TS[64:128, :], psx[64:128, :], p2c[64:128, sl], ALU.add)
            nc.vector.tensor_scalar(h1_2[64:128, sl], TS[64:128, :], 0.0, None, ALU.max)
        for b in range(NB):
            psh2 = ps_w.tile([64, RB], F32, tag="w")
            nc.tensor.matmul(psh2[:], ct['w1f2'][64:128, :],
                             h1_2[64:128, b * RB:(b + 1) * RB], start=True, stop=True)
            nc.vector.tensor_scalar(h2_2[0:64, b * RB:(b + 1) * RB], psh2[:],
                                    ct['bcol'][0:64, 6:7], 0.0, ALU.add, ALU.max)

        p2ps = ps_spl.tile([128, NCH, 32], F32, tag="spl")
        for i in range(NCH):
            nc.tensor.matmul(p2ps[:, i, :], h2_2[:, 128 * i:128 * (i + 1)], ct['wb2'][:],
                             start=True, stop=True)

        x31row = p_out.tile([128, NCH], F32, tag="x31row")
        pd2 = p_out.tile([128, NCH], F32, tag="pd2")
        spline(p2ps, x_row[:, :, 0:1], x31row, pd2, 2)

        # ---- outputs ----
        x3row = p_out.tile([128, NCH, 2], F32, tag="x3row")
        nc.vector.tensor_copy(x3row[:, :, 0], x31row[:])
        nc.vector.tensor_copy(x3row[:, :, 1], x22row[:])
        jam = p_out.tile([128, NCH], F32, tag="jam")
        nc.vector.tensor_tensor(jam[:], pd1[:], pd2[:], ALU.mult)
        jarow = p_out.tile([128, NCH], F32, tag="jarow")
        nc.scalar.activation(jarow[:], jam[:], AF.Abs)
        nc.gpsimd.dma_start(d_ox3[:, :, :], x3row[:])
        nc.gpsimd.dma_start(d_oja[:, :], jarow[:])

    nc.compile()
    return nc


# ---------------------------------------------------------------------------
# entry point
# ---------------------------------------------------------------------------
_CACHE = {}


def kernel(**inputs):
    from concourse.bass_utils import run_bass_kernel_spmd

    consts = build_consts(inputs)
    x = np.ascontiguousarray(np.asarray(inputs['x'], np.float32))
    c = np.ascontiguousarray(np.asarray(inputs['c'], np.float32))

    if 'nc' not in _CACHE:
        _CACHE['nc'] = build_program()
    nc = _CACHE['nc']

    in_maps = []
    for m in range(NCORES):
        xs = x[m * RC:(m + 1) * RC]
        cs = c[m * RC:(m + 1) * RC]
        br, bf = pack_blobs(consts, cs, xs)
        in_maps.append({'blob_r': br, 'blob_f': bf})

    res = run_bass_kernel_spmd(nc, in_maps, core_ids=list(range(NCORES)))
    x3 = np.empty((B, 2), np.float32)
    ja = np.empty((B, 1), np.float32)
    for m in range(NCORES):
        r = res.results[m]
        x3[m * RC:(m + 1) * RC] = r['o_x3'].transpose(1, 0, 2).reshape(RC, 2)
        ja[m * RC:(m + 1) * RC] = r['o_ja'].transpose(1, 0).reshape(RC, 1)
    return x3, ja


if __name__ == "__main__":
    nc = build_program()
    print("program built ok")
